# revision 13
# baseline (speedup 1.0000x reference)
"""Trainium2 Bass kernel for nn_Cluster_assigner (vq_codebook).

Sharding: data-parallel over batch bs=64 -> 8 cores x 8 batches.
Per core:
  phase 1 (per batch b):
    x_emb[b] (n,d) = x[b].T @ W.T + bias   via PE (stationary = x tiles (s,v))
    scores.T/P_u.T (n,c) fused as extra moving operand [G.T|H.T] where
      G = cemb @ W, H = chat @ W  (chat = l2norm(cluster_emb))
    norm2[n] = sum_d x_emb^2 (DVE scalar_tensor_tensor accum_out)
    expT = exp(scores.T * 1/sqrt(d))  (softmax w/o max-sub: scores ~ N(0,1))
    P_sum (n,c) += P_u.T * inv_norm   (+ const2 * sum_b inv_norm at the end)
    x_emb spilled to DRAM scratch (read back in phase 2)
  AllReduce P_sum (128KB) -> sinkhorn -> prob_avg (output 1) -> mask
  phase 2 (per batch): A.T (n,c) = expT * invE * mask ; attn-out accumulated
    into one PSUM bank across all batches; AllReduce (64KB) -> /64 (output 2)
"""

import math
import sys

import numpy as np

for _p in ("/opt/trn_rl_repo",):
    if _p not in sys.path:
        sys.path.insert(0, _p)

import concourse.bass as bass  # noqa: E402
import concourse.tile as tile  # noqa: E402
from concourse import bacc, mybir  # noqa: E402
from concourse import bass_utils  # noqa: E402
from concourse.masks import make_identity  # noqa: E402

F32 = mybir.dt.float32
AF = mybir.ActivationFunctionType
ALU = mybir.AluOpType

N_CORES = 8
BS = 64
BS_L = BS // N_CORES          # 8 batches per core
S = 1024                      # seq_len (contraction for x_emb)
V = 1024                      # n_vars (= n in the notes)
D = 512                       # d_model
C = 32                        # n_cluster
ST = S // 128                 # 8 s-tiles
VT = V // 128                 # 8 v-tiles
DT = D // 128                 # 4 d-tiles
EPS = 0.05
TEMP = 0.07
ATT_SCALE = 1.0 / math.sqrt(float(D))
SINK_SCALE = 1.0 / (BS * EPS)

TRACE = False
LAST_RESULTS = None


def _newton_rsqrt(nc, pool, inv, n2, shape, tag):
    """One Newton step refining inv ~= 1/sqrt(n2): inv *= (1.5 - 0.5*n2*inv^2)."""
    t = pool.tile(shape, F32, tag=tag)
    nc.vector.tensor_mul(t, inv, inv)
    nc.vector.tensor_mul(t, t, n2)
    # t = -0.5*t + 1.5  (Copy: out = in*scale + bias)
    nc.scalar.activation(t, t, AF.Copy, bias=1.5, scale=-0.5)
    nc.vector.tensor_mul(inv, inv, t)


def _body(tc, x_in, w_in, wt_in, ce_in, b_in, noise_in, pa_out, ca_out):
    nc = tc.nc

    from contextlib import ExitStack
    ctx = ExitStack()
    const = ctx.enter_context(tc.tile_pool(name="const", bufs=1))
    small = ctx.enter_context(tc.tile_pool(name="small", bufs=2))
    xpool = ctx.enter_context(tc.tile_pool(name="xp", bufs=2))
    xepool = ctx.enter_context(tc.tile_pool(name="xep", bufs=2))
    sppool = ctx.enter_context(tc.tile_pool(name="spp", bufs=2))
    persist = ctx.enter_context(tc.tile_pool(name="pers", bufs=1))
    dram = ctx.enter_context(tc.tile_pool(name="dram", bufs=1, space="DRAM"))

    # ---------------- constants / setup ----------------
    ident = const.tile([128, 128], F32)
    make_identity(nc, ident)
    wt_sb = const.tile([128, ST, D], F32)          # W.T tiles (s_p, s_t, d)
    nc.sync.dma_start(out=wt_sb, in_=wt_in.rearrange("(k p) d -> p k d", p=128))
    w_sb = const.tile([128, DT, S], F32)           # W tiles (d_p, d_t, s)
    nc.sync.dma_start(out=w_sb, in_=w_in.rearrange("(k p) s -> p k s", p=128))
    ce_sb = const.tile([C, D], F32)
    nc.sync.dma_start(out=ce_sb, in_=ce_in)
    b_row = const.tile([1, D], F32)
    nc.sync.dma_start(out=b_row, in_=b_in.rearrange("(a d) -> a d", a=1))
    b_col = const.tile([128, DT], F32)
    nc.sync.dma_start(out=b_col, in_=b_in.rearrange("(k p) -> p k", p=128))
    noise_sb = const.tile([128, VT, C], F32)
    nc.sync.dma_start(out=noise_sb, in_=noise_in.rearrange("(k p) c -> p k c", p=128))
    ones_row = const.tile([1, 128], F32)
    nc.vector.memset(ones_row, 1.0)
    ones_col = const.tile([128, 1], F32)
    nc.vector.memset(ones_col, 1.0)

    # chat = l2norm(cluster_emb)
    sq_c = small.tile([C, D], F32, tag="sqc")
    n2_c = small.tile([C, 1], F32, tag="n2c")
    nc.vector.scalar_tensor_tensor(
        out=sq_c, in0=ce_sb, scalar=1.0, in1=ce_sb,
        op0=ALU.mult, op1=ALU.mult, accum_out=n2_c)
    nrm_c = small.tile([C, 1], F32, tag="nrmc")
    nc.scalar.sqrt(nrm_c, n2_c)
    inv_c = small.tile([C, 1], F32, tag="invc")
    nc.vector.reciprocal(inv_c, nrm_c)
    _newton_rsqrt(nc, small, inv_c, n2_c, [C, 1], "newc")
    chat_sb = const.tile([C, D], F32)
    nc.vector.tensor_scalar_mul(chat_sb, ce_sb, inv_c)

    # ce2 = [cemb.T | chat.T]  (d_p, d_t, 2C)
    ce2_sb = const.tile([128, DT, 2 * C], F32)
    with tc.tile_pool(name="ps_setup", bufs=2, space="PSUM") as ps_setup:
        for dk in range(DT):
            pst = ps_setup.tile([128, 2 * C], F32, tag="tr")
            nc.tensor.transpose(pst[:, 0:C], ce_sb[:, dk * 128:(dk + 1) * 128],
                                ident[:C, :C])
            nc.tensor.transpose(pst[:, C:2 * C], chat_sb[:, dk * 128:(dk + 1) * 128],
                                ident[:C, :C])
            nc.vector.tensor_copy(ce2_sb[:, dk, :], pst)

        # GH.T (s_p, s_t, 2C): GH.T[s, :] = [G.T | H.T], G = cemb@W, H = chat@W
        ght_sb = const.tile([128, ST, 2 * C], F32)
        for st_i in range(ST):
            psg = ps_setup.tile([128, 2 * C], F32, tag="gh")
            for dk in range(DT):
                nc.tensor.matmul(
                    psg, w_sb[:, dk, st_i * 128:(st_i + 1) * 128], ce2_sb[:, dk, :],
                    start=(dk == 0), stop=(dk == DT - 1), skip_group_check=True)
            nc.vector.tensor_copy(ght_sb[:, st_i, :], psg)

        # const2[c] = chat @ b  (row layout (1, C))
        psc = ps_setup.tile([1, C], F32, tag="c2")
        for dk in range(DT):
            nc.tensor.matmul(psc, b_col[:, dk:dk + 1], ce2_sb[:, dk, C:2 * C],
                             start=(dk == 0), stop=(dk == DT - 1),
                             skip_group_check=True)
        const2_row = const.tile([1, C], F32)
        nc.vector.tensor_copy(const2_row, psc)
        # materialize across partitions (DVE can't read partition-stride-0;
        # stride-0 broadcast is only legal from DRAM, so bounce through it)
        c2_dram = dram.tile([1, C], F32)
        nc.sync.dma_start(out=c2_dram, in_=const2_row)
        const2_bc = const.tile([128, C], F32)
        nc.gpsimd.dma_start(out=const2_bc, in_=c2_dram.to_broadcast([128, C]))

    # DRAM scratch for x_emb and collective bounce buffers
    xe_dram = dram.tile([BS_L, VT, 128, D], F32)
    cc_p_in = dram.tile([128, VT, C], F32)
    cc_p_out = dram.tile([128, VT, C], F32)
    cc_a_in = dram.tile([C, D], F32)
    cc_a_out = dram.tile([C, D], F32)

    # persistent SBUF
    expT_sb = persist.tile([128, BS_L, VT, C], F32)    # exp(scores.T*scale)
    Psum_sb = persist.tile([128, VT, C], F32)          # sum_b P_u.T * inv_n
    Sinv_sb = persist.tile([128, VT], F32)             # sum_b inv_n (for const2)
    invE_sb = persist.tile([C, BS_L], F32)             # 1/softmax-denominator
    nc.vector.memset(Psum_sb, 0.0)
    nc.vector.memset(Sinv_sb, 0.0)

    ps_xe = ctx.enter_context(tc.tile_pool(name="ps_xe", bufs=2, space="PSUM"))
    ps_sp = ctx.enter_context(tc.tile_pool(name="ps_sp", bufs=2, space="PSUM"))
    ps_e = ctx.enter_context(tc.tile_pool(name="ps_e", bufs=2, space="PSUM"))

    # ---------------- phase 1 ----------------
    for b in range(BS_L):
        x_sb = xpool.tile([128, ST, V], F32, tag="x")
        nc.sync.dma_start(out=x_sb, in_=x_in[b].rearrange("(k p) v -> p k v", p=128))
        xe_sb = xepool.tile([128, VT, D], F32, tag="xe")
        spT_sb = sppool.tile([128, VT, 2 * C], F32, tag="sp")
        n2_sb = small.tile([128, VT], F32, tag="n2")

        for vt in range(VT):
            pxe = ps_xe.tile([128, D], F32, tag="xe")
            psp = ps_sp.tile([128, 2 * C], F32, tag="sp")
            for st_i in range(ST):
                lhsT = x_sb[:, st_i, vt * 128:(vt + 1) * 128]
                nc.tensor.matmul(pxe, lhsT, wt_sb[:, st_i, :],
                                 start=(st_i == 0), stop=False,
                                 skip_group_check=True)
                nc.tensor.matmul(psp, lhsT, ght_sb[:, st_i, :],
                                 start=(st_i == 0), stop=(st_i == ST - 1),
                                 skip_group_check=True)
            # bias add via K=1 matmul: xe += ones_col(v) * b_row(d)
            nc.tensor.matmul(pxe, ones_row, b_row, start=False, stop=True,
                             skip_group_check=True)
            nc.vector.tensor_copy(xe_sb[:, vt, :], pxe)
            nc.vector.tensor_copy(spT_sb[:, vt, :], psp)
            # norm2 = sum_d xe^2 (one DVE op w/ accumulate output)
            sq_t = small.tile([128, D], F32, tag="sq")
            nc.vector.scalar_tensor_tensor(
                out=sq_t, in0=xe_sb[:, vt, :], scalar=1.0, in1=xe_sb[:, vt, :],
                op0=ALU.mult, op1=ALU.mult, accum_out=n2_sb[:, vt:vt + 1])
            # exp of scaled scores (no max-subtraction; scores ~ N(0,1))
            nc.scalar.activation(expT_sb[:, b, vt, :], spT_sb[:, vt, 0:C],
                                 AF.Exp, scale=ATT_SCALE)

        # spill x_emb to DRAM
        nc.sync.dma_start(out=xe_dram[b].rearrange("k p d -> p k d"), in_=xe_sb)

        # inv_n = 1/sqrt(norm2), one Newton step
        nrm = small.tile([128, VT], F32, tag="nrm")
        nc.scalar.sqrt(nrm, n2_sb)
        invn = small.tile([128, VT], F32, tag="invn")
        nc.vector.reciprocal(invn, nrm)
        _newton_rsqrt(nc, small, invn, n2_sb, [128, VT], "newt")
        nc.vector.tensor_add(Sinv_sb, Sinv_sb, invn)

        # softmax denominators in column layout: E[c] = sum_n expT
        # (expT tile as stationary, ones as moving -> out (C, 1))
        pse = ps_e.tile([C, 1], F32, tag="E")
        for vt in range(VT):
            nc.tensor.matmul(pse, expT_sb[:, b, vt, :], ones_col,
                             start=(vt == 0), stop=(vt == VT - 1),
                             skip_group_check=True)
        esum = small.tile([C, 1], F32, tag="es")
        nc.vector.tensor_copy(esum, pse)
        nc.vector.reciprocal(invE_sb[:, b:b + 1], esum)

        # P_sum += P_u.T * inv_n   (per-partition scalar multiply, fused add)
        for vt in range(VT):
            nc.vector.scalar_tensor_tensor(
                out=Psum_sb[:, vt, :], in0=spT_sb[:, vt, C:2 * C],
                scalar=invn[:, vt:vt + 1], in1=Psum_sb[:, vt, :],
                op0=ALU.mult, op1=ALU.add)

    # finish P_sum: += const2 * Sinv  (bias term, zero when b == 0)
    for vt in range(VT):
        nc.vector.scalar_tensor_tensor(
            out=Psum_sb[:, vt, :], in0=const2_bc,
            scalar=Sinv_sb[:, vt:vt + 1], in1=Psum_sb[:, vt, :],
            op0=ALU.mult, op1=ALU.add)

    # ---------------- all-reduce P_sum ----------------
    nc.sync.dma_start(out=cc_p_in, in_=Psum_sb)
    nc.gpsimd.collective_compute(
        "AllReduce", ALU.add, replica_groups=[list(range(N_CORES))],
        ins=[cc_p_in.opt()], outs=[cc_p_out.opt()])
    Pg_sb = persist.tile([128, VT, C], F32)
    nc.sync.dma_start(out=Pg_sb, in_=cc_p_out)

    # ---------------- sinkhorn -> prob_avg (output 1) ----------------
    Q_sb = persist.tile([128, VT, C], F32)
    rs = small.tile([128, VT], F32, tag="rs")
    for vt in range(VT):
        nc.scalar.activation(Q_sb[:, vt, :], Pg_sb[:, vt, :], AF.Exp,
                             scale=SINK_SCALE, accum_out=rs[:, vt:vt + 1])
    irs = small.tile([128, VT], F32, tag="irs")
    nc.vector.reciprocal(irs, rs)
    probavg_sb = persist.tile([128, VT, C], F32)
    for vt in range(VT):
        nc.vector.tensor_scalar_mul(probavg_sb[:, vt, :], Q_sb[:, vt, :],
                                    irs[:, vt:vt + 1])
    nc.sync.dma_start(out=pa_out.rearrange("(k p) c -> p k c", p=128),
                      in_=probavg_sb)

    # ---------------- mask ----------------
    c_eps = small.tile([128, 1], F32, tag="ceps")
    nc.vector.memset(c_eps, 1e-10)
    c_1eps = small.tile([128, 1], F32, tag="c1eps")
    nc.vector.memset(c_1eps, 1.0 + 1e-10)
    c_one = small.tile([128, 1], F32, tag="cone")
    nc.vector.memset(c_one, 1.0)
    lp1 = small.tile([128, VT, C], F32, tag="lp1")
    nc.scalar.activation(lp1, probavg_sb, AF.Ln, bias=c_eps[:, 0:1])
    lp2 = small.tile([128, VT, C], F32, tag="lp2")
    nc.scalar.activation(lp2, probavg_sb, AF.Ln, bias=c_1eps[:, 0:1], scale=-1.0)
    ln1 = small.tile([128, VT, C], F32, tag="ln1")
    nc.scalar.activation(ln1, noise_sb, AF.Ln)
    ln2 = small.tile([128, VT, C], F32, tag="ln2")
    nc.scalar.activation(ln2, noise_sb, AF.Ln, bias=c_one[:, 0:1], scale=-1.0)
    nc.vector.tensor_sub(lp1, lp1, lp2)
    nc.vector.tensor_sub(ln1, ln1, ln2)
    nc.vector.tensor_add(lp1, lp1, ln1)
    mask_sb = persist.tile([128, VT, C], F32)
    nc.scalar.activation(mask_sb, lp1, AF.Sigmoid, scale=1.0 / TEMP)

    # ---------------- phase 2: attention ----------------
    osum = small.tile([C, D], F32, tag="os")
    nc.vector.memset(osum, 0.0)
    with tc.tile_pool(name="ps_out", bufs=2, space="PSUM") as ps_out:
        for b in range(BS_L):
            at_sb = small.tile([128, VT, C], F32, tag="at")
            nc.vector.tensor_mul(at_sb, expT_sb[:, b, :, :], mask_sb)
            xeb = xepool.tile([128, VT, D], F32, tag="xe")
            nc.sync.dma_start(out=xeb, in_=xe_dram[b].rearrange("k p d -> p k d"))
            pso = ps_out.tile([C, D], F32, tag="o")
            for vt in range(VT):
                nc.tensor.matmul(pso, at_sb[:, vt, :], xeb[:, vt, :],
                                 start=(vt == 0), stop=(vt == VT - 1),
                                 skip_group_check=True)
            # osum += pso * invE[b]  (per-partition scalar on cluster axis)
            nc.vector.scalar_tensor_tensor(
                out=osum, in0=pso, scalar=invE_sb[:, b:b + 1], in1=osum,
                op0=ALU.mult, op1=ALU.add)

    nc.sync.dma_start(out=cc_a_in, in_=osum)
    nc.gpsimd.collective_compute(
        "AllReduce", ALU.add, replica_groups=[list(range(N_CORES))],
        ins=[cc_a_in.opt()], outs=[cc_a_out.opt()])
    gsum = small.tile([C, D], F32, tag="gs")
    nc.sync.dma_start(out=gsum, in_=cc_a_out)
    final = small.tile([C, D], F32, tag="fin")
    nc.scalar.activation(final, gsum, AF.Copy, scale=1.0 / BS)
    nc.sync.dma_start(out=ca_out, in_=final)
    ctx.close()


_NC_CACHE = {}


def _get_nc():
    if "nc" not in _NC_CACHE:
        nc = bacc.Bacc("TRN2", target_bir_lowering=False, debug=False,
                       enable_asserts=False, num_devices=N_CORES)
        x_in = nc.dram_tensor("x_l", [BS_L, S, V], F32, kind="ExternalInput").ap()
        w_in = nc.dram_tensor("w", [D, S], F32, kind="ExternalInput").ap()
        wt_in = nc.dram_tensor("wt", [S, D], F32, kind="ExternalInput").ap()
        ce_in = nc.dram_tensor("cemb", [C, D], F32, kind="ExternalInput").ap()
        b_in = nc.dram_tensor("bvec", [D], F32, kind="ExternalInput").ap()
        noise_in = nc.dram_tensor("noise", [V, C], F32, kind="ExternalInput").ap()
        pa_out = nc.dram_tensor("prob_avg", [V, C], F32, kind="ExternalOutput").ap()
        ca_out = nc.dram_tensor("cluster_avg", [C, D], F32,
                                kind="ExternalOutput").ap()
        with tile.TileContext(nc) as tc:
            _body(tc, x_in, w_in, wt_in, ce_in, b_in, noise_in, pa_out, ca_out)
        nc.compile()
        _NC_CACHE["nc"] = nc
    return _NC_CACHE["nc"]


def kernel(x, cluster_emb, W, b, noise):
    global LAST_RESULTS
    nc = _get_nc()
    x = np.ascontiguousarray(np.asarray(x, dtype=np.float32))
    W = np.ascontiguousarray(np.asarray(W, dtype=np.float32))
    wt = np.ascontiguousarray(W.T)
    ce = np.ascontiguousarray(np.asarray(cluster_emb, dtype=np.float32))
    bv = np.ascontiguousarray(np.asarray(b, dtype=np.float32))
    nz = np.ascontiguousarray(np.asarray(noise, dtype=np.float32))
    in_maps = []
    for core in range(N_CORES):
        in_maps.append({
            "x_l": np.ascontiguousarray(x[core * BS_L:(core + 1) * BS_L]),
            "w": W, "wt": wt, "cemb": ce, "bvec": bv, "noise": nz,
        })
    res = bass_utils.run_bass_kernel_spmd(
        nc, in_maps, core_ids=list(range(N_CORES)), trace=TRACE)
    LAST_RESULTS = res
    r0 = res.results[0]
    return (r0["prob_avg"].copy(), r0["cluster_avg"].copy())


# revision 16
# speedup vs baseline: 2.5700x; 2.5700x over previous
"""Trainium2 Bass kernel for nn_Cluster_assigner (vq_codebook).

Sharding: data-parallel over batch bs=64 -> 8 cores x 8 batches.
Per core:
  phase 1 (per batch b):
    x_emb[b] (n,d) = x[b].T @ W.T + bias   via PE (stationary = x tiles (s,v))
    scores.T/P_u.T (n,c) fused as extra moving operand [G.T|H.T] where
      G = cemb @ W, H = chat @ W  (chat = l2norm(cluster_emb))
    norm2[n] = sum_d x_emb^2 (DVE scalar_tensor_tensor accum_out)
    expT = exp(scores.T * 1/sqrt(d))  (softmax w/o max-sub: scores ~ N(0,1))
    P_sum (n,c) += P_u.T * inv_norm   (+ const2 * sum_b inv_norm at the end)
    x_emb spilled to DRAM scratch (read back in phase 2)
  AllReduce P_sum (128KB) -> sinkhorn -> prob_avg (output 1) -> mask
  phase 2 (per batch): A.T (n,c) = expT * invE * mask ; attn-out accumulated
    into one PSUM bank across all batches; AllReduce (64KB) -> /64 (output 2)
"""

import math
import sys

import numpy as np

for _p in ("/opt/trn_rl_repo",):
    if _p not in sys.path:
        sys.path.insert(0, _p)

import concourse.bass as bass  # noqa: E402
import concourse.tile as tile  # noqa: E402
from concourse import bacc, mybir  # noqa: E402
from concourse import bass_utils  # noqa: E402
from concourse.masks import make_identity  # noqa: E402

F32 = mybir.dt.float32
BF16 = mybir.dt.bfloat16
AF = mybir.ActivationFunctionType
ALU = mybir.AluOpType

N_CORES = 8
BS = 64
BS_L = BS // N_CORES          # 8 batches per core
S = 1024                      # seq_len (contraction for x_emb)
V = 1024                      # n_vars (= n in the notes)
D = 512                       # d_model
C = 32                        # n_cluster
ST = S // 128                 # 8 s-tiles
VT = V // 128                 # 8 v-tiles
DT = D // 128                 # 4 d-tiles
EPS = 0.05
TEMP = 0.07
ATT_SCALE = 1.0 / math.sqrt(float(D))
SINK_SCALE = 1.0 / (BS * EPS)

TRACE = False
LAST_RESULTS = None


def _newton_rsqrt(nc, pool, inv, n2, shape, tag):
    """One Newton step refining inv ~= 1/sqrt(n2): inv *= (1.5 - 0.5*n2*inv^2)."""
    t = pool.tile(shape, F32, tag=tag)
    nc.vector.tensor_mul(t, inv, inv)
    nc.vector.tensor_mul(t, t, n2)
    # t = -0.5*t + 1.5  (Copy: out = in*scale + bias)
    nc.scalar.activation(t, t, AF.Copy, bias=1.5, scale=-0.5)
    nc.vector.tensor_mul(inv, inv, t)


def _body(tc, x_in, w_in, wt_in, ce_in, b_in, noise_in, pa_out, ca_out):
    nc = tc.nc

    from contextlib import ExitStack
    ctx = ExitStack()
    const = ctx.enter_context(tc.tile_pool(name="const", bufs=1))
    small = ctx.enter_context(tc.tile_pool(name="small", bufs=2))
    xpool = ctx.enter_context(tc.tile_pool(name="xp", bufs=2))
    xepool = ctx.enter_context(tc.tile_pool(name="xep", bufs=2))
    sppool = ctx.enter_context(tc.tile_pool(name="spp", bufs=2))
    persist = ctx.enter_context(tc.tile_pool(name="pers", bufs=1))
    dram = ctx.enter_context(tc.tile_pool(name="dram", bufs=1, space="DRAM"))

    # ---------------- constants / setup ----------------
    ident = const.tile([128, 128], F32)
    make_identity(nc, ident)
    wt_sb = const.tile([128, ST, D], BF16)         # W.T tiles (s_p, s_t, d), bf16
    nc.gpsimd.dma_start(out=wt_sb, in_=wt_in.rearrange("(k p) d -> p k d", p=128))
    w_sb = const.tile([128, DT, S], F32)           # W tiles (d_p, d_t, s)
    nc.sync.dma_start(out=w_sb, in_=w_in.rearrange("(k p) s -> p k s", p=128))
    ce_sb = const.tile([C, D], F32)
    nc.sync.dma_start(out=ce_sb, in_=ce_in)
    b_row = const.tile([1, D], BF16)
    nc.gpsimd.dma_start(out=b_row, in_=b_in.rearrange("(a d) -> a d", a=1))
    b_col = const.tile([128, DT], F32)
    nc.sync.dma_start(out=b_col, in_=b_in.rearrange("(k p) -> p k", p=128))
    noise_sb = const.tile([128, VT, C], F32)
    nc.sync.dma_start(out=noise_sb, in_=noise_in.rearrange("(k p) c -> p k c", p=128))
    ones_row = const.tile([1, 128], BF16)
    nc.vector.memset(ones_row, 1.0)
    ones_col = const.tile([128, 1], BF16)
    nc.vector.memset(ones_col, 1.0)

    # chat = l2norm(cluster_emb)
    sq_c = small.tile([C, D], F32, tag="sqc")
    n2_c = small.tile([C, 1], F32, tag="n2c")
    nc.vector.scalar_tensor_tensor(
        out=sq_c, in0=ce_sb, scalar=1.0, in1=ce_sb,
        op0=ALU.mult, op1=ALU.mult, accum_out=n2_c)
    nrm_c = small.tile([C, 1], F32, tag="nrmc")
    nc.scalar.sqrt(nrm_c, n2_c)
    inv_c = small.tile([C, 1], F32, tag="invc")
    nc.vector.reciprocal(inv_c, nrm_c)
    _newton_rsqrt(nc, small, inv_c, n2_c, [C, 1], "newc")
    chat_sb = const.tile([C, D], F32)
    nc.vector.tensor_scalar_mul(chat_sb, ce_sb, inv_c)

    # ce2 = [cemb.T | chat.T]  (d_p, d_t, 2C)
    ce2_sb = const.tile([128, DT, 2 * C], F32)
    with tc.tile_pool(name="ps_setup", bufs=2, space="PSUM") as ps_setup:
        for dk in range(DT):
            pst = ps_setup.tile([128, 2 * C], F32, tag="tr")
            nc.tensor.transpose(pst[:, 0:C], ce_sb[:, dk * 128:(dk + 1) * 128],
                                ident[:C, :C])
            nc.tensor.transpose(pst[:, C:2 * C], chat_sb[:, dk * 128:(dk + 1) * 128],
                                ident[:C, :C])
            nc.vector.tensor_copy(ce2_sb[:, dk, :], pst)

        # GH.T (s_p, s_t, 2C): GH.T[s, :] = [G.T | H.T], G = cemb@W, H = chat@W
        ght_sb = const.tile([128, ST, 2 * C], BF16)
        for st_i in range(ST):
            psg = ps_setup.tile([128, 2 * C], F32, tag="gh")
            for dk in range(DT):
                nc.tensor.matmul(
                    psg, w_sb[:, dk, st_i * 128:(st_i + 1) * 128], ce2_sb[:, dk, :],
                    start=(dk == 0), stop=(dk == DT - 1), skip_group_check=True)
            nc.vector.tensor_copy(ght_sb[:, st_i, :], psg)

        # const2[c] = chat @ b  (row layout (1, C))
        psc = ps_setup.tile([1, C], F32, tag="c2")
        for dk in range(DT):
            nc.tensor.matmul(psc, b_col[:, dk:dk + 1], ce2_sb[:, dk, C:2 * C],
                             start=(dk == 0), stop=(dk == DT - 1),
                             skip_group_check=True)
        const2_row = const.tile([1, C], F32)
        nc.vector.tensor_copy(const2_row, psc)
        # materialize across partitions (DVE can't read partition-stride-0;
        # stride-0 broadcast is only legal from DRAM, so bounce through it)
        c2_dram = dram.tile([1, C], F32)
        nc.sync.dma_start(out=c2_dram, in_=const2_row)
        const2_bc = const.tile([128, C], F32)
        nc.gpsimd.dma_start(out=const2_bc, in_=c2_dram.to_broadcast([128, C]))

    # DRAM scratch for x_emb and collective bounce buffers
    xe_dram = dram.tile([BS_L, VT, 128, D], BF16)
    cc_p_in = dram.tile([128, VT, C], F32)
    cc_p_out = dram.tile([128, VT, C], F32)
    cc_a_in = dram.tile([C, D], F32)
    cc_a_out = dram.tile([C, D], F32)

    # persistent SBUF
    expT_sb = persist.tile([128, BS_L, VT, C], BF16)   # exp(scores.T*scale)
    PuT_all = persist.tile([128, BS_L, VT, C], F32)    # unnormalized prob.T
    n2_all = persist.tile([128, BS_L, VT], F32)        # row norms^2 of x_emb
    Psum_sb = persist.tile([128, VT, C], F32)          # sum_b P_u.T * inv_n
    Sinv_sb = persist.tile([128, VT], F32)             # sum_b inv_n (for const2)
    invE_sb = persist.tile([C, BS_L], F32)             # 1/softmax-denominator
    nc.vector.memset(Psum_sb, 0.0)
    nc.vector.memset(Sinv_sb, 0.0)

    ps_xe = ctx.enter_context(tc.tile_pool(name="ps_xe", bufs=2, space="PSUM"))
    ps_sp = ctx.enter_context(tc.tile_pool(name="ps_sp", bufs=2, space="PSUM"))
    ps_e = ctx.enter_context(tc.tile_pool(name="ps_e", bufs=2, space="PSUM"))

    # ---------------- phase 1 ----------------
    for b in range(BS_L):
        x_sb = xpool.tile([128, ST, V], BF16, tag="x")
        nc.gpsimd.dma_start(out=x_sb, in_=x_in[b].rearrange("(k p) v -> p k v", p=128))
        xe_sb = xepool.tile([128, VT, D], BF16, tag="xe")

        for vt in range(VT):
            pxe = ps_xe.tile([128, D], F32, tag="xe")
            psp = ps_sp.tile([128, 2 * C], F32, tag="sp")
            for st_i in range(ST):
                lhsT = x_sb[:, st_i, vt * 128:(vt + 1) * 128]
                nc.tensor.matmul(pxe, lhsT, wt_sb[:, st_i, :],
                                 start=(st_i == 0), stop=False,
                                 skip_group_check=True)
                nc.tensor.matmul(psp, lhsT, ght_sb[:, st_i, :],
                                 start=(st_i == 0), stop=(st_i == ST - 1),
                                 skip_group_check=True)
            # bias add via K=1 matmul: xe += ones_col(v) * b_row(d)
            nc.tensor.matmul(pxe, ones_row, b_row, start=False, stop=True,
                             skip_group_check=True)
            nc.vector.tensor_copy(xe_sb[:, vt, :], pxe)
            # norm2 = sum_d xe^2 straight from PSUM (fp32 accurate).
            # ACT Square: PSUM has a single DVE read port, so a 2-operand
            # DVE op reading pxe twice is illegal; ScalarE reads it once.
            sq_t = small.tile([128, D], F32, tag="sq")
            nc.scalar.activation(sq_t, pxe, AF.Square,
                                 accum_out=n2_all[:, b, vt:vt + 1])
            # keep unnormalized prob.T; exp(scores.T) straight from PSUM
            nc.vector.tensor_copy(PuT_all[:, b, vt, :], psp[:, C:2 * C])
            nc.scalar.activation(expT_sb[:, b, vt, :], psp[:, 0:C],
                                 AF.Exp, scale=ATT_SCALE)

        # spill x_emb to DRAM
        nc.sync.dma_start(out=xe_dram[b].rearrange("k p d -> p k d"), in_=xe_sb)

        # softmax denominators in column layout: E[c] = sum_n expT
        # (expT tile as stationary, ones as moving -> out (C, 1))
        pse = ps_e.tile([C, 1], F32, tag="E")
        for vt in range(VT):
            nc.tensor.matmul(pse, expT_sb[:, b, vt, :], ones_col,
                             start=(vt == 0), stop=(vt == VT - 1),
                             skip_group_check=True)
        esum = small.tile([C, 1], F32, tag="es")
        nc.vector.tensor_copy(esum, pse)
        nc.vector.reciprocal(invE_sb[:, b:b + 1], esum)

    # ---- deferred norms + P accumulation (keeps ACT on one table set) ----
    nrm = small.tile([128, BS_L, VT], F32, tag="nrm")
    nc.scalar.sqrt(nrm, n2_all)
    invn = small.tile([128, BS_L, VT], F32, tag="invn")
    nc.vector.reciprocal(invn, nrm)
    _newton_rsqrt(nc, small, invn, n2_all, [128, BS_L, VT], "newt")
    for b in range(BS_L):
        nc.vector.tensor_add(Sinv_sb, Sinv_sb, invn[:, b, :])
        for vt in range(VT):
            nc.vector.scalar_tensor_tensor(
                out=Psum_sb[:, vt, :], in0=PuT_all[:, b, vt, :],
                scalar=invn[:, b, vt:vt + 1], in1=Psum_sb[:, vt, :],
                op0=ALU.mult, op1=ALU.add)
    # finish P_sum: += const2 * Sinv  (bias term, zero when b == 0)
    for vt in range(VT):
        nc.vector.scalar_tensor_tensor(
            out=Psum_sb[:, vt, :], in0=const2_bc,
            scalar=Sinv_sb[:, vt:vt + 1], in1=Psum_sb[:, vt, :],
            op0=ALU.mult, op1=ALU.add)

    # ---------------- all-reduce P_sum ----------------
    nc.sync.dma_start(out=cc_p_in, in_=Psum_sb)
    nc.gpsimd.collective_compute(
        "AllReduce", ALU.add, replica_groups=[list(range(N_CORES))],
        ins=[cc_p_in.opt()], outs=[cc_p_out.opt()])
    Pg_sb = persist.tile([128, VT, C], F32)
    nc.sync.dma_start(out=Pg_sb, in_=cc_p_out)

    # ---------------- sinkhorn -> prob_avg (output 1) ----------------
    Q_sb = persist.tile([128, VT, C], F32)
    rs = small.tile([128, VT], F32, tag="rs")
    for vt in range(VT):
        nc.scalar.activation(Q_sb[:, vt, :], Pg_sb[:, vt, :], AF.Exp,
                             scale=SINK_SCALE, accum_out=rs[:, vt:vt + 1])
    irs = small.tile([128, VT], F32, tag="irs")
    nc.vector.reciprocal(irs, rs)
    probavg_sb = persist.tile([128, VT, C], F32)
    for vt in range(VT):
        nc.vector.tensor_scalar_mul(probavg_sb[:, vt, :], Q_sb[:, vt, :],
                                    irs[:, vt:vt + 1])
    nc.sync.dma_start(out=pa_out.rearrange("(k p) c -> p k c", p=128),
                      in_=probavg_sb)

    # ---------------- mask ----------------
    c_eps = small.tile([128, 1], F32, tag="ceps")
    nc.vector.memset(c_eps, 1e-10)
    c_1eps = small.tile([128, 1], F32, tag="c1eps")
    nc.vector.memset(c_1eps, 1.0 + 1e-10)
    c_one = small.tile([128, 1], F32, tag="cone")
    nc.vector.memset(c_one, 1.0)
    lp1 = small.tile([128, VT, C], F32, tag="lp1")
    nc.scalar.activation(lp1, probavg_sb, AF.Ln, bias=c_eps[:, 0:1])
    lp2 = small.tile([128, VT, C], F32, tag="lp2")
    nc.scalar.activation(lp2, probavg_sb, AF.Ln, bias=c_1eps[:, 0:1], scale=-1.0)
    ln1 = small.tile([128, VT, C], F32, tag="ln1")
    nc.scalar.activation(ln1, noise_sb, AF.Ln)
    ln2 = small.tile([128, VT, C], F32, tag="ln2")
    nc.scalar.activation(ln2, noise_sb, AF.Ln, bias=c_one[:, 0:1], scale=-1.0)
    nc.vector.tensor_sub(lp1, lp1, lp2)
    nc.vector.tensor_sub(ln1, ln1, ln2)
    nc.vector.tensor_add(lp1, lp1, ln1)
    mask_sb = persist.tile([128, VT, C], BF16)
    nc.scalar.activation(mask_sb, lp1, AF.Sigmoid, scale=1.0 / TEMP)

    # ---------------- phase 2: attention ----------------
    osum = small.tile([C, D], F32, tag="os")
    nc.vector.memset(osum, 0.0)
    with tc.tile_pool(name="ps_out", bufs=2, space="PSUM") as ps_out:
        for b in range(BS_L):
            at_sb = small.tile([128, VT, C], BF16, tag="at")
            nc.vector.tensor_mul(at_sb, expT_sb[:, b, :, :], mask_sb)
            xeb = xepool.tile([128, VT, D], BF16, tag="xe")
            nc.sync.dma_start(out=xeb, in_=xe_dram[b].rearrange("k p d -> p k d"))
            pso = ps_out.tile([C, D], F32, tag="o")
            for vt in range(VT):
                nc.tensor.matmul(pso, at_sb[:, vt, :], xeb[:, vt, :],
                                 start=(vt == 0), stop=(vt == VT - 1),
                                 skip_group_check=True)
            # osum += pso * invE[b]  (per-partition scalar on cluster axis)
            nc.vector.scalar_tensor_tensor(
                out=osum, in0=pso, scalar=invE_sb[:, b:b + 1], in1=osum,
                op0=ALU.mult, op1=ALU.add)

    nc.sync.dma_start(out=cc_a_in, in_=osum)
    nc.gpsimd.collective_compute(
        "AllReduce", ALU.add, replica_groups=[list(range(N_CORES))],
        ins=[cc_a_in.opt()], outs=[cc_a_out.opt()])
    gsum = small.tile([C, D], F32, tag="gs")
    nc.sync.dma_start(out=gsum, in_=cc_a_out)
    final = small.tile([C, D], F32, tag="fin")
    nc.scalar.activation(final, gsum, AF.Copy, scale=1.0 / BS)
    nc.sync.dma_start(out=ca_out, in_=final)
    ctx.close()


_NC_CACHE = {}


def _get_nc():
    if "nc" not in _NC_CACHE:
        nc = bacc.Bacc("TRN2", target_bir_lowering=False, debug=False,
                       enable_asserts=False, num_devices=N_CORES)
        x_in = nc.dram_tensor("x_l", [BS_L, S, V], F32, kind="ExternalInput").ap()
        w_in = nc.dram_tensor("w", [D, S], F32, kind="ExternalInput").ap()
        wt_in = nc.dram_tensor("wt", [S, D], F32, kind="ExternalInput").ap()
        ce_in = nc.dram_tensor("cemb", [C, D], F32, kind="ExternalInput").ap()
        b_in = nc.dram_tensor("bvec", [D], F32, kind="ExternalInput").ap()
        noise_in = nc.dram_tensor("noise", [V, C], F32, kind="ExternalInput").ap()
        pa_out = nc.dram_tensor("prob_avg", [V, C], F32, kind="ExternalOutput").ap()
        ca_out = nc.dram_tensor("cluster_avg", [C, D], F32,
                                kind="ExternalOutput").ap()
        with tile.TileContext(nc) as tc:
            _body(tc, x_in, w_in, wt_in, ce_in, b_in, noise_in, pa_out, ca_out)
        nc.compile()
        _NC_CACHE["nc"] = nc
    return _NC_CACHE["nc"]


def kernel(x, cluster_emb, W, b, noise):
    global LAST_RESULTS
    nc = _get_nc()
    x = np.ascontiguousarray(np.asarray(x, dtype=np.float32))
    W = np.ascontiguousarray(np.asarray(W, dtype=np.float32))
    wt = np.ascontiguousarray(W.T)
    ce = np.ascontiguousarray(np.asarray(cluster_emb, dtype=np.float32))
    bv = np.ascontiguousarray(np.asarray(b, dtype=np.float32))
    nz = np.ascontiguousarray(np.asarray(noise, dtype=np.float32))
    in_maps = []
    for core in range(N_CORES):
        in_maps.append({
            "x_l": np.ascontiguousarray(x[core * BS_L:(core + 1) * BS_L]),
            "w": W, "wt": wt, "cemb": ce, "bvec": bv, "noise": nz,
        })
    res = bass_utils.run_bass_kernel_spmd(
        nc, in_maps, core_ids=list(range(N_CORES)), trace=TRACE)
    LAST_RESULTS = res
    r0 = res.results[0]
    return (r0["prob_avg"].copy(), r0["cluster_avg"].copy())


# revision 17
# speedup vs baseline: 2.7406x; 1.0664x over previous
"""Trainium2 Bass kernel for nn_Cluster_assigner (vq_codebook).

Sharding: data-parallel over batch bs=64 -> 8 cores x 8 batches.
Per core:
  phase 1 (per batch b):
    x_emb[b] (n,d) = x[b].T @ W.T + bias   via PE (stationary = x tiles (s,v))
    scores.T/P_u.T (n,c) fused as extra moving operand [G.T|H.T] where
      G = cemb @ W, H = chat @ W  (chat = l2norm(cluster_emb))
    norm2[n] = sum_d x_emb^2 (DVE scalar_tensor_tensor accum_out)
    expT = exp(scores.T * 1/sqrt(d))  (softmax w/o max-sub: scores ~ N(0,1))
    P_sum (n,c) += P_u.T * inv_norm   (+ const2 * sum_b inv_norm at the end)
    x_emb spilled to DRAM scratch (read back in phase 2)
  AllReduce P_sum (128KB) -> sinkhorn -> prob_avg (output 1) -> mask
  phase 2 (per batch): A.T (n,c) = expT * invE * mask ; attn-out accumulated
    into one PSUM bank across all batches; AllReduce (64KB) -> /64 (output 2)
"""

import math
import sys

import numpy as np

for _p in ("/opt/trn_rl_repo",):
    if _p not in sys.path:
        sys.path.insert(0, _p)

import concourse.bass as bass  # noqa: E402
import concourse.tile as tile  # noqa: E402
from concourse import bacc, mybir  # noqa: E402
from concourse import bass_utils  # noqa: E402
from concourse.masks import make_identity  # noqa: E402

F32 = mybir.dt.float32
BF16 = mybir.dt.bfloat16
AF = mybir.ActivationFunctionType
ALU = mybir.AluOpType

N_CORES = 8
BS = 64
BS_L = BS // N_CORES          # 8 batches per core
S = 1024                      # seq_len (contraction for x_emb)
V = 1024                      # n_vars (= n in the notes)
D = 512                       # d_model
C = 32                        # n_cluster
ST = S // 128                 # 8 s-tiles
VT = V // 128                 # 8 v-tiles
DT = D // 128                 # 4 d-tiles
EPS = 0.05
TEMP = 0.07
ATT_SCALE = 1.0 / math.sqrt(float(D))
SINK_SCALE = 1.0 / (BS * EPS)

TRACE = False
LAST_RESULTS = None


def _newton_rsqrt(nc, pool, inv, n2, shape, tag):
    """One Newton step refining inv ~= 1/sqrt(n2): inv *= (1.5 - 0.5*n2*inv^2)."""
    t = pool.tile(shape, F32, tag=tag)
    nc.vector.tensor_mul(t, inv, inv)
    nc.vector.tensor_mul(t, t, n2)
    # t = -0.5*t + 1.5  (Copy: out = in*scale + bias)
    nc.scalar.activation(t, t, AF.Copy, bias=1.5, scale=-0.5)
    nc.vector.tensor_mul(inv, inv, t)


def _body(tc, x_in, w_in, wt_in, ce_in, b_in, noise_in, pa_out, ca_out):
    nc = tc.nc

    from contextlib import ExitStack
    ctx = ExitStack()
    const = ctx.enter_context(tc.tile_pool(name="const", bufs=1))
    small = ctx.enter_context(tc.tile_pool(name="small", bufs=2))
    xpool = ctx.enter_context(tc.tile_pool(name="xp", bufs=2))
    xepool = ctx.enter_context(tc.tile_pool(name="xep", bufs=4))
    sppool = ctx.enter_context(tc.tile_pool(name="spp", bufs=2))
    persist = ctx.enter_context(tc.tile_pool(name="pers", bufs=1))
    dram = ctx.enter_context(tc.tile_pool(name="dram", bufs=1, space="DRAM"))

    # ---------------- first x tile load (ahead of setup DMAs) ----------
    x_tiles = {}

    def load_x(b):
        t = xpool.tile([128, ST, V], BF16, tag="x")
        nc.gpsimd.dma_start(out=t, in_=x_in[b].rearrange("(k p) v -> p k v", p=128))
        x_tiles[b] = t

    load_x(0)

    # ---------------- constants / setup ----------------
    ident = const.tile([128, 128], F32)
    make_identity(nc, ident)
    wt_sb = const.tile([128, ST, D], BF16)         # W.T tiles (s_p, s_t, d), bf16
    nc.gpsimd.dma_start(out=wt_sb, in_=wt_in.rearrange("(k p) d -> p k d", p=128))
    w_sb = const.tile([128, DT, S], F32)           # W tiles (d_p, d_t, s)
    nc.sync.dma_start(out=w_sb, in_=w_in.rearrange("(k p) s -> p k s", p=128))
    ce_sb = const.tile([C, D], F32)
    nc.sync.dma_start(out=ce_sb, in_=ce_in)
    b_row = const.tile([1, D], BF16)
    nc.gpsimd.dma_start(out=b_row, in_=b_in.rearrange("(a d) -> a d", a=1))
    b_col = const.tile([128, DT], F32)
    nc.sync.dma_start(out=b_col, in_=b_in.rearrange("(k p) -> p k", p=128))
    noise_sb = const.tile([128, VT, C], F32)
    nc.sync.dma_start(out=noise_sb, in_=noise_in.rearrange("(k p) c -> p k c", p=128))
    ones_row = const.tile([1, 128], BF16)
    nc.vector.memset(ones_row, 1.0)
    ones_col = const.tile([128, 1], BF16)
    nc.vector.memset(ones_col, 1.0)

    # chat = l2norm(cluster_emb)
    sq_c = small.tile([C, D], F32, tag="sqc")
    n2_c = small.tile([C, 1], F32, tag="n2c")
    nc.vector.scalar_tensor_tensor(
        out=sq_c, in0=ce_sb, scalar=1.0, in1=ce_sb,
        op0=ALU.mult, op1=ALU.mult, accum_out=n2_c)
    nrm_c = small.tile([C, 1], F32, tag="nrmc")
    nc.scalar.sqrt(nrm_c, n2_c)
    inv_c = small.tile([C, 1], F32, tag="invc")
    nc.vector.reciprocal(inv_c, nrm_c)
    _newton_rsqrt(nc, small, inv_c, n2_c, [C, 1], "newc")
    chat_sb = const.tile([C, D], F32)
    nc.vector.tensor_scalar_mul(chat_sb, ce_sb, inv_c)

    # ce2 = [cemb.T | chat.T]  (d_p, d_t, 2C)
    ce2_sb = const.tile([128, DT, 2 * C], F32)
    with tc.tile_pool(name="ps_setup", bufs=2, space="PSUM") as ps_setup:
        for dk in range(DT):
            pst = ps_setup.tile([128, 2 * C], F32, tag="tr")
            nc.tensor.transpose(pst[:, 0:C], ce_sb[:, dk * 128:(dk + 1) * 128],
                                ident[:C, :C])
            nc.tensor.transpose(pst[:, C:2 * C], chat_sb[:, dk * 128:(dk + 1) * 128],
                                ident[:C, :C])
            nc.vector.tensor_copy(ce2_sb[:, dk, :], pst)

        # GH.T (s_p, s_t, 2C): GH.T[s, :] = [G.T | H.T], G = cemb@W, H = chat@W
        ght_sb = const.tile([128, ST, 2 * C], BF16)
        for st_i in range(ST):
            psg = ps_setup.tile([128, 2 * C], F32, tag="gh")
            for dk in range(DT):
                nc.tensor.matmul(
                    psg, w_sb[:, dk, st_i * 128:(st_i + 1) * 128], ce2_sb[:, dk, :],
                    start=(dk == 0), stop=(dk == DT - 1), skip_group_check=True)
            nc.vector.tensor_copy(ght_sb[:, st_i, :], psg)

        # const2[c] = chat @ b  (row layout (1, C))
        psc = ps_setup.tile([1, C], F32, tag="c2")
        for dk in range(DT):
            nc.tensor.matmul(psc, b_col[:, dk:dk + 1], ce2_sb[:, dk, C:2 * C],
                             start=(dk == 0), stop=(dk == DT - 1),
                             skip_group_check=True)
        const2_row = const.tile([1, C], F32)
        nc.vector.tensor_copy(const2_row, psc)
        # materialize across partitions (DVE can't read partition-stride-0;
        # stride-0 broadcast is only legal from DRAM, so bounce through it)
        c2_dram = dram.tile([1, C], F32)
        nc.sync.dma_start(out=c2_dram, in_=const2_row)
        const2_bc = const.tile([128, C], F32)
        nc.gpsimd.dma_start(out=const2_bc, in_=c2_dram.to_broadcast([128, C]))

    # DRAM scratch for x_emb and collective bounce buffers
    xe_dram = dram.tile([BS_L, VT, 128, D], BF16)
    cc_p_in = dram.tile([128, VT, C], F32)
    cc_p_out = dram.tile([128, VT, C], F32)
    cc_a_in = dram.tile([C, D], F32)
    cc_a_out = dram.tile([C, D], F32)

    # persistent SBUF
    expT_sb = persist.tile([128, BS_L, VT, C], BF16)   # exp(scores.T*scale)
    PuT_all = persist.tile([128, BS_L, VT, C], F32)    # unnormalized prob.T
    n2_all = persist.tile([128, BS_L, VT], F32)        # row norms^2 of x_emb
    Psum_sb = persist.tile([128, VT, C], F32)          # sum_b P_u.T * inv_n
    Sinv_sb = persist.tile([128, VT], F32)             # sum_b inv_n (for const2)
    invE_sb = persist.tile([C, BS_L], F32)             # 1/softmax-denominator
    nc.vector.memset(Psum_sb, 0.0)
    nc.vector.memset(Sinv_sb, 0.0)

    ps_xe = ctx.enter_context(tc.tile_pool(name="ps_xe", bufs=2, space="PSUM"))
    ps_sp = ctx.enter_context(tc.tile_pool(name="ps_sp", bufs=2, space="PSUM"))
    ps_e = ctx.enter_context(tc.tile_pool(name="ps_e", bufs=2, space="PSUM"))

    # ---------------- phase 1 ----------------
    for b in range(BS_L):
        if b not in x_tiles:
            load_x(b)
        x_sb = x_tiles.pop(b)
        xe_sb = xepool.tile([128, VT, D], BF16, tag="xe")

        for vt in range(VT):
            pxe = ps_xe.tile([128, D], F32, tag="xe")
            psp = ps_sp.tile([128, 2 * C], F32, tag="sp")
            for st_i in range(ST):
                lhsT = x_sb[:, st_i, vt * 128:(vt + 1) * 128]
                nc.tensor.matmul(pxe, lhsT, wt_sb[:, st_i, :],
                                 start=(st_i == 0), stop=False,
                                 skip_group_check=True)
                nc.tensor.matmul(psp, lhsT, ght_sb[:, st_i, :],
                                 start=(st_i == 0), stop=(st_i == ST - 1),
                                 skip_group_check=True)
            # bias add via K=1 matmul: xe += ones_col(v) * b_row(d)
            nc.tensor.matmul(pxe, ones_row, b_row, start=False, stop=True,
                             skip_group_check=True)
            nc.vector.tensor_copy(xe_sb[:, vt, :], pxe)
            # norm2 = sum_d xe^2 straight from PSUM (fp32 accurate).
            # ACT Square: PSUM has a single DVE read port, so a 2-operand
            # DVE op reading pxe twice is illegal; ScalarE reads it once.
            sq_t = small.tile([128, D], F32, tag="sq")
            nc.scalar.activation(sq_t, pxe, AF.Square,
                                 accum_out=n2_all[:, b, vt:vt + 1])
            # keep unnormalized prob.T; exp(scores.T) straight from PSUM
            nc.vector.tensor_copy(PuT_all[:, b, vt, :], psp[:, C:2 * C])
            nc.scalar.activation(expT_sb[:, b, vt, :], psp[:, 0:C],
                                 AF.Exp, scale=ATT_SCALE)

        # spill x_emb to DRAM
        nc.sync.dma_start(out=xe_dram[b].rearrange("k p d -> p k d"), in_=xe_sb)

        # softmax denominators in column layout: E[c] = sum_n expT
        # (expT tile as stationary, ones as moving -> out (C, 1))
        pse = ps_e.tile([C, 1], F32, tag="E")
        for vt in range(VT):
            nc.tensor.matmul(pse, expT_sb[:, b, vt, :], ones_col,
                             start=(vt == 0), stop=(vt == VT - 1),
                             skip_group_check=True)
        esum = small.tile([C, 1], F32, tag="es")
        nc.vector.tensor_copy(esum, pse)
        nc.vector.reciprocal(invE_sb[:, b:b + 1], esum)

        # norms + P accumulation per batch (hidden under PE)
        nrm = small.tile([128, VT], F32, tag="nrm")
        nc.scalar.sqrt(nrm, n2_all[:, b, :])
        invn = small.tile([128, VT], F32, tag="invn")
        nc.vector.reciprocal(invn, nrm)
        _newton_rsqrt(nc, small, invn, n2_all[:, b, :], [128, VT], "newt")
        nc.vector.tensor_add(Sinv_sb, Sinv_sb, invn)
        for vt in range(VT):
            nc.vector.scalar_tensor_tensor(
                out=Psum_sb[:, vt, :], in0=PuT_all[:, b, vt, :],
                scalar=invn[:, vt:vt + 1], in1=Psum_sb[:, vt, :],
                op0=ALU.mult, op1=ALU.add)
    # finish P_sum: += const2 * Sinv  (bias term, zero when b == 0)
    for vt in range(VT):
        nc.vector.scalar_tensor_tensor(
            out=Psum_sb[:, vt, :], in0=const2_bc,
            scalar=Sinv_sb[:, vt:vt + 1], in1=Psum_sb[:, vt, :],
            op0=ALU.mult, op1=ALU.add)

    # ---------------- all-reduce P_sum ----------------
    nc.sync.dma_start(out=cc_p_in, in_=Psum_sb)
    nc.gpsimd.collective_compute(
        "AllReduce", ALU.add, replica_groups=[list(range(N_CORES))],
        ins=[cc_p_in.opt()], outs=[cc_p_out.opt()])
    Pg_sb = persist.tile([128, VT, C], F32)
    nc.sync.dma_start(out=Pg_sb, in_=cc_p_out)

    # ---------------- sinkhorn -> prob_avg (output 1) ----------------
    Q_sb = persist.tile([128, VT, C], F32)
    rs = small.tile([128, VT], F32, tag="rs")
    for vt in range(VT):
        nc.scalar.activation(Q_sb[:, vt, :], Pg_sb[:, vt, :], AF.Exp,
                             scale=SINK_SCALE, accum_out=rs[:, vt:vt + 1])
    irs = small.tile([128, VT], F32, tag="irs")
    nc.vector.reciprocal(irs, rs)
    probavg_sb = persist.tile([128, VT, C], F32)
    for vt in range(VT):
        nc.vector.tensor_scalar_mul(probavg_sb[:, vt, :], Q_sb[:, vt, :],
                                    irs[:, vt:vt + 1])
    nc.sync.dma_start(out=pa_out.rearrange("(k p) c -> p k c", p=128),
                      in_=probavg_sb)

    # ---------------- mask ----------------
    c_eps = small.tile([128, 1], F32, tag="ceps")
    nc.vector.memset(c_eps, 1e-10)
    c_1eps = small.tile([128, 1], F32, tag="c1eps")
    nc.vector.memset(c_1eps, 1.0 + 1e-10)
    c_one = small.tile([128, 1], F32, tag="cone")
    nc.vector.memset(c_one, 1.0)
    lp1 = small.tile([128, VT, C], F32, tag="lp1")
    nc.scalar.activation(lp1, probavg_sb, AF.Ln, bias=c_eps[:, 0:1])
    lp2 = small.tile([128, VT, C], F32, tag="lp2")
    nc.scalar.activation(lp2, probavg_sb, AF.Ln, bias=c_1eps[:, 0:1], scale=-1.0)
    ln1 = small.tile([128, VT, C], F32, tag="ln1")
    nc.scalar.activation(ln1, noise_sb, AF.Ln)
    ln2 = small.tile([128, VT, C], F32, tag="ln2")
    nc.scalar.activation(ln2, noise_sb, AF.Ln, bias=c_one[:, 0:1], scale=-1.0)
    nc.vector.tensor_sub(lp1, lp1, lp2)
    nc.vector.tensor_sub(ln1, ln1, ln2)
    nc.vector.tensor_add(lp1, lp1, ln1)
    mask_sb = persist.tile([128, VT, C], BF16)
    nc.scalar.activation(mask_sb, lp1, AF.Sigmoid, scale=1.0 / TEMP)

    # ---------------- phase 2: attention ----------------
    osum = small.tile([C, D], F32, tag="os")
    nc.vector.memset(osum, 0.0)
    with tc.tile_pool(name="ps_out", bufs=2, space="PSUM") as ps_out:
        for b in range(BS_L):
            at_sb = small.tile([128, VT, C], BF16, tag="at")
            nc.vector.tensor_mul(at_sb, expT_sb[:, b, :, :], mask_sb)
            xeb = xepool.tile([128, VT, D], BF16, tag="xe")
            nc.sync.dma_start(out=xeb, in_=xe_dram[b].rearrange("k p d -> p k d"))
            pso = ps_out.tile([C, D], F32, tag="o")
            for vt in range(VT):
                nc.tensor.matmul(pso, at_sb[:, vt, :], xeb[:, vt, :],
                                 start=(vt == 0), stop=(vt == VT - 1),
                                 skip_group_check=True)
            # osum += pso * invE[b]  (per-partition scalar on cluster axis)
            nc.vector.scalar_tensor_tensor(
                out=osum, in0=pso, scalar=invE_sb[:, b:b + 1], in1=osum,
                op0=ALU.mult, op1=ALU.add)

    nc.sync.dma_start(out=cc_a_in, in_=osum)
    nc.gpsimd.collective_compute(
        "AllReduce", ALU.add, replica_groups=[list(range(N_CORES))],
        ins=[cc_a_in.opt()], outs=[cc_a_out.opt()])
    gsum = small.tile([C, D], F32, tag="gs")
    nc.sync.dma_start(out=gsum, in_=cc_a_out)
    final = small.tile([C, D], F32, tag="fin")
    nc.scalar.activation(final, gsum, AF.Copy, scale=1.0 / BS)
    nc.sync.dma_start(out=ca_out, in_=final)
    ctx.close()


_NC_CACHE = {}


def _get_nc():
    if "nc" not in _NC_CACHE:
        nc = bacc.Bacc("TRN2", target_bir_lowering=False, debug=False,
                       enable_asserts=False, num_devices=N_CORES)
        x_in = nc.dram_tensor("x_l", [BS_L, S, V], F32, kind="ExternalInput").ap()
        w_in = nc.dram_tensor("w", [D, S], F32, kind="ExternalInput").ap()
        wt_in = nc.dram_tensor("wt", [S, D], F32, kind="ExternalInput").ap()
        ce_in = nc.dram_tensor("cemb", [C, D], F32, kind="ExternalInput").ap()
        b_in = nc.dram_tensor("bvec", [D], F32, kind="ExternalInput").ap()
        noise_in = nc.dram_tensor("noise", [V, C], F32, kind="ExternalInput").ap()
        pa_out = nc.dram_tensor("prob_avg", [V, C], F32, kind="ExternalOutput").ap()
        ca_out = nc.dram_tensor("cluster_avg", [C, D], F32,
                                kind="ExternalOutput").ap()
        with tile.TileContext(nc) as tc:
            _body(tc, x_in, w_in, wt_in, ce_in, b_in, noise_in, pa_out, ca_out)
        nc.compile()
        _NC_CACHE["nc"] = nc
    return _NC_CACHE["nc"]


def kernel(x, cluster_emb, W, b, noise):
    global LAST_RESULTS
    nc = _get_nc()
    x = np.ascontiguousarray(np.asarray(x, dtype=np.float32))
    W = np.ascontiguousarray(np.asarray(W, dtype=np.float32))
    wt = np.ascontiguousarray(W.T)
    ce = np.ascontiguousarray(np.asarray(cluster_emb, dtype=np.float32))
    bv = np.ascontiguousarray(np.asarray(b, dtype=np.float32))
    nz = np.ascontiguousarray(np.asarray(noise, dtype=np.float32))
    in_maps = []
    for core in range(N_CORES):
        in_maps.append({
            "x_l": np.ascontiguousarray(x[core * BS_L:(core + 1) * BS_L]),
            "w": W, "wt": wt, "cemb": ce, "bvec": bv, "noise": nz,
        })
    res = bass_utils.run_bass_kernel_spmd(
        nc, in_maps, core_ids=list(range(N_CORES)), trace=TRACE)
    LAST_RESULTS = res
    r0 = res.results[0]
    return (r0["prob_avg"].copy(), r0["cluster_avg"].copy())


# revision 18
# speedup vs baseline: 3.1708x; 1.1570x over previous
"""Trainium2 Bass kernel for nn_Cluster_assigner (vq_codebook).

Sharding: data-parallel over batch bs=64 -> 8 cores x 8 batches.
Per core:
  phase 1 (per batch b):
    x_emb[b] (n,d) = x[b].T @ W.T + bias   via PE (stationary = x tiles (s,v))
    scores.T/P_u.T (n,c) fused as extra moving operand [G.T|H.T] where
      G = cemb @ W, H = chat @ W  (chat = l2norm(cluster_emb))
    norm2[n] = sum_d x_emb^2 (DVE scalar_tensor_tensor accum_out)
    expT = exp(scores.T * 1/sqrt(d))  (softmax w/o max-sub: scores ~ N(0,1))
    P_sum (n,c) += P_u.T * inv_norm   (+ const2 * sum_b inv_norm at the end)
    x_emb spilled to DRAM scratch (read back in phase 2)
  AllReduce P_sum (128KB) -> sinkhorn -> prob_avg (output 1) -> mask
  phase 2 (per batch): A.T (n,c) = expT * invE * mask ; attn-out accumulated
    into one PSUM bank across all batches; AllReduce (64KB) -> /64 (output 2)
"""

import math
import sys

import numpy as np

for _p in ("/opt/trn_rl_repo",):
    if _p not in sys.path:
        sys.path.insert(0, _p)

import concourse.bass as bass  # noqa: E402
import concourse.tile as tile  # noqa: E402
from concourse import bacc, mybir  # noqa: E402
from concourse import bass_utils  # noqa: E402
from concourse.masks import make_identity  # noqa: E402

F32 = mybir.dt.float32
BF16 = mybir.dt.bfloat16
AF = mybir.ActivationFunctionType
ALU = mybir.AluOpType

N_CORES = 8
BS = 64
BS_L = BS // N_CORES          # 8 batches per core
S = 1024                      # seq_len (contraction for x_emb)
V = 1024                      # n_vars (= n in the notes)
D = 512                       # d_model
C = 32                        # n_cluster
ST = S // 128                 # 8 s-tiles
VT = V // 128                 # 8 v-tiles
DT = D // 128                 # 4 d-tiles
EPS = 0.05
TEMP = 0.07
ATT_SCALE = 1.0 / math.sqrt(float(D))
SINK_SCALE = 1.0 / (BS * EPS)

TRACE = False
LAST_RESULTS = None


def _newton_rsqrt(nc, pool, inv, n2, shape, tag):
    """One Newton step refining inv ~= 1/sqrt(n2): inv *= (1.5 - 0.5*n2*inv^2)."""
    t = pool.tile(shape, F32, tag=tag)
    nc.vector.tensor_mul(t, inv, inv)
    nc.vector.tensor_mul(t, t, n2)
    # t = -0.5*t + 1.5  (Copy: out = in*scale + bias)
    nc.scalar.activation(t, t, AF.Copy, bias=1.5, scale=-0.5)
    nc.vector.tensor_mul(inv, inv, t)


def _body(tc, x_in, w_in, wt_in, ce_in, b_in, noise_in, pa_out, ca_out,
          with_bias):
    nc = tc.nc

    from contextlib import ExitStack
    ctx = ExitStack()
    const = ctx.enter_context(tc.tile_pool(name="const", bufs=1))
    small = ctx.enter_context(tc.tile_pool(name="small", bufs=2))
    xpool = ctx.enter_context(tc.tile_pool(name="xp", bufs=2))
    xepool = ctx.enter_context(tc.tile_pool(name="xep", bufs=4))
    sppool = ctx.enter_context(tc.tile_pool(name="spp", bufs=2))
    persist = ctx.enter_context(tc.tile_pool(name="pers", bufs=1))
    dram = ctx.enter_context(tc.tile_pool(name="dram", bufs=1, space="DRAM"))

    # ---------------- first x tile load (ahead of setup DMAs) ----------
    x_tiles = {}

    def load_x(b):
        t = xpool.tile([128, ST, V], BF16, tag="x")
        nc.gpsimd.dma_start(out=t, in_=x_in[b].rearrange("(k p) v -> p k v", p=128))
        x_tiles[b] = t

    load_x(0)

    # ---------------- constants / setup ----------------
    ident = const.tile([128, 128], F32)
    make_identity(nc, ident)
    wt_sb = const.tile([128, ST, D], BF16)         # W.T tiles (s_p, s_t, d), bf16
    nc.gpsimd.dma_start(out=wt_sb, in_=wt_in.rearrange("(k p) d -> p k d", p=128))
    w_sb = const.tile([128, DT, S], F32)           # W tiles (d_p, d_t, s)
    nc.sync.dma_start(out=w_sb, in_=w_in.rearrange("(k p) s -> p k s", p=128))
    ce_sb = const.tile([C, D], F32)
    nc.sync.dma_start(out=ce_sb, in_=ce_in)
    if with_bias:
        b_row = const.tile([1, D], BF16)
        nc.gpsimd.dma_start(out=b_row, in_=b_in.rearrange("(a d) -> a d", a=1))
        b_col = const.tile([128, DT], F32)
        nc.sync.dma_start(out=b_col, in_=b_in.rearrange("(k p) -> p k", p=128))
    noise_sb = const.tile([128, VT, C], F32)
    nc.sync.dma_start(out=noise_sb, in_=noise_in.rearrange("(k p) c -> p k c", p=128))
    ones_row = const.tile([1, 128], BF16)
    nc.vector.memset(ones_row, 1.0)
    ones_col = const.tile([128, 1], BF16)
    nc.vector.memset(ones_col, 1.0)

    # chat = l2norm(cluster_emb)
    sq_c = small.tile([C, D], F32, tag="sqc")
    n2_c = small.tile([C, 1], F32, tag="n2c")
    nc.vector.scalar_tensor_tensor(
        out=sq_c, in0=ce_sb, scalar=1.0, in1=ce_sb,
        op0=ALU.mult, op1=ALU.mult, accum_out=n2_c)
    nrm_c = small.tile([C, 1], F32, tag="nrmc")
    nc.scalar.sqrt(nrm_c, n2_c)
    inv_c = small.tile([C, 1], F32, tag="invc")
    nc.vector.reciprocal(inv_c, nrm_c)
    _newton_rsqrt(nc, small, inv_c, n2_c, [C, 1], "newc")
    chat_sb = const.tile([C, D], F32)
    nc.vector.tensor_scalar_mul(chat_sb, ce_sb, inv_c)

    # ce2 = [cemb.T | chat.T]  (d_p, d_t, 2C)
    ce2_sb = const.tile([128, DT, 2 * C], F32)
    with tc.tile_pool(name="ps_setup", bufs=2, space="PSUM") as ps_setup:
        for dk in range(DT):
            pst = ps_setup.tile([128, 2 * C], F32, tag="tr")
            nc.tensor.transpose(pst[:, 0:C], ce_sb[:, dk * 128:(dk + 1) * 128],
                                ident[:C, :C])
            nc.tensor.transpose(pst[:, C:2 * C], chat_sb[:, dk * 128:(dk + 1) * 128],
                                ident[:C, :C])
            nc.vector.tensor_copy(ce2_sb[:, dk, :], pst)

        # GH.T (s_p, s_t, 2C): GH.T[s, :] = [G.T | H.T], G = cemb@W, H = chat@W
        ght_sb = const.tile([128, ST, 2 * C], BF16)
        for st_i in range(ST):
            psg = ps_setup.tile([128, 2 * C], F32, tag="gh")
            for dk in range(DT):
                nc.tensor.matmul(
                    psg, w_sb[:, dk, st_i * 128:(st_i + 1) * 128], ce2_sb[:, dk, :],
                    start=(dk == 0), stop=(dk == DT - 1), skip_group_check=True)
            nc.vector.tensor_copy(ght_sb[:, st_i, :], psg)

        if with_bias:
            # const2[c] = chat @ b  (row layout (1, C))
            psc = ps_setup.tile([1, C], F32, tag="c2")
            for dk in range(DT):
                nc.tensor.matmul(psc, b_col[:, dk:dk + 1],
                                 ce2_sb[:, dk, C:2 * C],
                                 start=(dk == 0), stop=(dk == DT - 1),
                                 skip_group_check=True)
            const2_row = const.tile([1, C], F32)
            nc.vector.tensor_copy(const2_row, psc)
            # materialize across partitions (stride-0 only legal from DRAM)
            c2_dram = dram.tile([1, C], F32)
            nc.sync.dma_start(out=c2_dram, in_=const2_row)
            const2_bc = const.tile([128, C], F32)
            nc.gpsimd.dma_start(out=const2_bc, in_=c2_dram.to_broadcast([128, C]))

    # DRAM scratch for x_emb and collective bounce buffers
    xe_dram = dram.tile([BS_L, VT, 128, D], BF16)
    cc_p_in = dram.tile([128, VT, C], F32)
    cc_p_out = dram.tile([128, VT, C], F32)
    cc_a_in = dram.tile([C, D], F32)
    cc_a_out = dram.tile([C, D], F32)

    # persistent SBUF
    expT_sb = persist.tile([128, BS_L, VT, C], BF16)   # exp(scores.T*scale)
    PuT_all = persist.tile([128, BS_L, VT, C], F32)    # unnormalized prob.T
    n2_all = persist.tile([128, BS_L, VT], F32)        # row norms^2 of x_emb
    Psum_sb = persist.tile([128, VT, C], F32)          # sum_b P_u.T * inv_n
    Sinv_sb = persist.tile([128, VT], F32)             # sum_b inv_n (for const2)
    invE_sb = persist.tile([C, BS_L], F32)             # 1/softmax-denominator
    nc.vector.memset(Psum_sb, 0.0)
    nc.vector.memset(Sinv_sb, 0.0)

    ps_xe = ctx.enter_context(tc.tile_pool(name="ps_xe", bufs=2, space="PSUM"))
    ps_sp = ctx.enter_context(tc.tile_pool(name="ps_sp", bufs=2, space="PSUM"))
    ps_e = ctx.enter_context(tc.tile_pool(name="ps_e", bufs=2, space="PSUM"))

    # ---------------- phase 1 ----------------
    for b in range(BS_L):
        if b not in x_tiles:
            load_x(b)
        x_sb = x_tiles.pop(b)
        xe_sb = xepool.tile([128, VT, D], BF16, tag="xe")

        for vt in range(VT):
            pxe = ps_xe.tile([128, D], F32, tag="xe")
            psp = ps_sp.tile([128, 2 * C], F32, tag="sp")
            for st_i in range(ST):
                lhsT = x_sb[:, st_i, vt * 128:(vt + 1) * 128]
                nc.tensor.matmul(pxe, lhsT, wt_sb[:, st_i, :],
                                 start=(st_i == 0),
                                 stop=(not with_bias and st_i == ST - 1),
                                 skip_group_check=True)
                nc.tensor.matmul(psp, lhsT, ght_sb[:, st_i, :],
                                 start=(st_i == 0), stop=(st_i == ST - 1),
                                 skip_group_check=True)
            if with_bias:
                # bias add via K=1 matmul: xe += ones_col(v) * b_row(d)
                nc.tensor.matmul(pxe, ones_row, b_row, start=False, stop=True,
                                 skip_group_check=True)
            nc.vector.tensor_copy(xe_sb[:, vt, :], pxe)
            # norm2 = sum_d xe^2 straight from PSUM (fp32 accurate).
            # ACT Square: PSUM has a single DVE read port, so a 2-operand
            # DVE op reading pxe twice is illegal; ScalarE reads it once.
            sq_t = small.tile([128, D], F32, tag="sq")
            nc.scalar.activation(sq_t, pxe, AF.Square,
                                 accum_out=n2_all[:, b, vt:vt + 1])
            # keep unnormalized prob.T; exp(scores.T) straight from PSUM
            nc.vector.tensor_copy(PuT_all[:, b, vt, :], psp[:, C:2 * C])
            nc.scalar.activation(expT_sb[:, b, vt, :], psp[:, 0:C],
                                 AF.Exp, scale=ATT_SCALE)

        # spill x_emb to DRAM
        nc.sync.dma_start(out=xe_dram[b].rearrange("k p d -> p k d"), in_=xe_sb)

        # softmax denominators in column layout: E[c] = sum_n expT
        # (expT tile as stationary, ones as moving -> out (C, 1))
        pse = ps_e.tile([C, 1], F32, tag="E")
        for vt in range(VT):
            nc.tensor.matmul(pse, expT_sb[:, b, vt, :], ones_col,
                             start=(vt == 0), stop=(vt == VT - 1),
                             skip_group_check=True)
        esum = small.tile([C, 1], F32, tag="es")
        nc.vector.tensor_copy(esum, pse)
        nc.vector.reciprocal(invE_sb[:, b:b + 1], esum)

        # norms + P accumulation per batch (hidden under PE)
        nrm = small.tile([128, VT], F32, tag="nrm")
        nc.scalar.sqrt(nrm, n2_all[:, b, :])
        invn = small.tile([128, VT], F32, tag="invn")
        nc.vector.reciprocal(invn, nrm)
        _newton_rsqrt(nc, small, invn, n2_all[:, b, :], [128, VT], "newt")
        if with_bias:
            nc.vector.tensor_add(Sinv_sb, Sinv_sb, invn)
        for vt in range(VT):
            nc.vector.scalar_tensor_tensor(
                out=Psum_sb[:, vt, :], in0=PuT_all[:, b, vt, :],
                scalar=invn[:, vt:vt + 1], in1=Psum_sb[:, vt, :],
                op0=ALU.mult, op1=ALU.add)
    if with_bias:
        # finish P_sum: += const2 * Sinv
        for vt in range(VT):
            nc.vector.scalar_tensor_tensor(
                out=Psum_sb[:, vt, :], in0=const2_bc,
                scalar=Sinv_sb[:, vt:vt + 1], in1=Psum_sb[:, vt, :],
                op0=ALU.mult, op1=ALU.add)

    # ---------------- all-reduce P_sum ----------------
    nc.sync.dma_start(out=cc_p_in, in_=Psum_sb)
    nc.gpsimd.collective_compute(
        "AllReduce", ALU.add, replica_groups=[list(range(N_CORES))],
        ins=[cc_p_in.opt()], outs=[cc_p_out.opt()])
    Pg_sb = persist.tile([128, VT, C], F32)
    nc.sync.dma_start(out=Pg_sb, in_=cc_p_out)

    # ---------------- sinkhorn -> prob_avg (output 1) ----------------
    Q_sb = persist.tile([128, VT, C], F32)
    rs = small.tile([128, VT], F32, tag="rs")
    for vt in range(VT):
        nc.scalar.activation(Q_sb[:, vt, :], Pg_sb[:, vt, :], AF.Exp,
                             scale=SINK_SCALE, accum_out=rs[:, vt:vt + 1])
    irs = small.tile([128, VT], F32, tag="irs")
    nc.vector.reciprocal(irs, rs)
    probavg_sb = persist.tile([128, VT, C], F32)
    for vt in range(VT):
        nc.vector.tensor_scalar_mul(probavg_sb[:, vt, :], Q_sb[:, vt, :],
                                    irs[:, vt:vt + 1])
    nc.sync.dma_start(out=pa_out.rearrange("(k p) c -> p k c", p=128),
                      in_=probavg_sb)

    # ---------------- mask ----------------
    c_eps = small.tile([128, 1], F32, tag="ceps")
    nc.vector.memset(c_eps, 1e-10)
    c_1eps = small.tile([128, 1], F32, tag="c1eps")
    nc.vector.memset(c_1eps, 1.0 + 1e-10)
    c_one = small.tile([128, 1], F32, tag="cone")
    nc.vector.memset(c_one, 1.0)
    lp1 = small.tile([128, VT, C], F32, tag="lp1")
    nc.scalar.activation(lp1, probavg_sb, AF.Ln, bias=c_eps[:, 0:1])
    lp2 = small.tile([128, VT, C], F32, tag="lp2")
    nc.scalar.activation(lp2, probavg_sb, AF.Ln, bias=c_1eps[:, 0:1], scale=-1.0)
    nc.vector.tensor_sub(lp1, lp1, lp2)
    nc.vector.tensor_add(lp1, lp1, noise_sb)  # noise_sb = host logit_noise
    mask_sb = persist.tile([128, VT, C], BF16)
    nc.scalar.activation(mask_sb, lp1, AF.Sigmoid, scale=1.0 / TEMP)

    # ---------------- phase 2: attention ----------------
    osum = small.tile([C, D], F32, tag="os")
    nc.vector.memset(osum, 0.0)
    with tc.tile_pool(name="ps_out", bufs=2, space="PSUM") as ps_out:
        for b in range(BS_L):
            at_sb = small.tile([128, VT, C], BF16, tag="at")
            nc.vector.tensor_mul(at_sb, expT_sb[:, b, :, :], mask_sb)
            xeb = xepool.tile([128, VT, D], BF16, tag="xe")
            nc.sync.dma_start(out=xeb, in_=xe_dram[b].rearrange("k p d -> p k d"))
            pso = ps_out.tile([C, D], F32, tag="o")
            for vt in range(VT):
                nc.tensor.matmul(pso, at_sb[:, vt, :], xeb[:, vt, :],
                                 start=(vt == 0), stop=(vt == VT - 1),
                                 skip_group_check=True)
            # osum += pso * invE[b]  (per-partition scalar on cluster axis)
            nc.vector.scalar_tensor_tensor(
                out=osum, in0=pso, scalar=invE_sb[:, b:b + 1], in1=osum,
                op0=ALU.mult, op1=ALU.add)

    nc.sync.dma_start(out=cc_a_in, in_=osum)
    nc.gpsimd.collective_compute(
        "AllReduce", ALU.add, replica_groups=[list(range(N_CORES))],
        ins=[cc_a_in.opt()], outs=[cc_a_out.opt()])
    gsum = small.tile([C, D], F32, tag="gs")
    nc.sync.dma_start(out=gsum, in_=cc_a_out)
    final = small.tile([C, D], F32, tag="fin")
    nc.scalar.activation(final, gsum, AF.Copy, scale=1.0 / BS)
    nc.sync.dma_start(out=ca_out, in_=final)
    ctx.close()


_NC_CACHE = {}


def _get_nc(with_bias=False):
    if with_bias not in _NC_CACHE:
        nc = bacc.Bacc("TRN2", target_bir_lowering=False, debug=False,
                       enable_asserts=False, num_devices=N_CORES)
        x_in = nc.dram_tensor("x_l", [BS_L, S, V], F32, kind="ExternalInput").ap()
        w_in = nc.dram_tensor("w", [D, S], F32, kind="ExternalInput").ap()
        wt_in = nc.dram_tensor("wt", [S, D], F32, kind="ExternalInput").ap()
        ce_in = nc.dram_tensor("cemb", [C, D], F32, kind="ExternalInput").ap()
        b_in = nc.dram_tensor("bvec", [D], F32, kind="ExternalInput").ap()
        noise_in = nc.dram_tensor("noise", [V, C], F32, kind="ExternalInput").ap()
        pa_out = nc.dram_tensor("prob_avg", [V, C], F32, kind="ExternalOutput").ap()
        ca_out = nc.dram_tensor("cluster_avg", [C, D], F32,
                                kind="ExternalOutput").ap()
        with tile.TileContext(nc) as tc:
            _body(tc, x_in, w_in, wt_in, ce_in, b_in, noise_in, pa_out, ca_out,
                  with_bias)
        nc.compile()
        _NC_CACHE[with_bias] = nc
    return _NC_CACHE[with_bias]


def kernel(x, cluster_emb, W, b, noise):
    global LAST_RESULTS
    bv = np.ascontiguousarray(np.asarray(b, dtype=np.float32))
    nc = _get_nc(bool(np.any(bv)))
    x = np.ascontiguousarray(np.asarray(x, dtype=np.float32))
    W = np.ascontiguousarray(np.asarray(W, dtype=np.float32))
    wt = np.ascontiguousarray(W.T)
    ce = np.ascontiguousarray(np.asarray(cluster_emb, dtype=np.float32))
    nzf = np.asarray(noise, dtype=np.float32)
    nz = np.ascontiguousarray(np.log(nzf) - np.log1p(-nzf))
    in_maps = []
    for core in range(N_CORES):
        in_maps.append({
            "x_l": np.ascontiguousarray(x[core * BS_L:(core + 1) * BS_L]),
            "w": W, "wt": wt, "cemb": ce, "bvec": bv, "noise": nz,
        })
    res = bass_utils.run_bass_kernel_spmd(
        nc, in_maps, core_ids=list(range(N_CORES)), trace=TRACE)
    LAST_RESULTS = res
    r0 = res.results[0]
    return (r0["prob_avg"].copy(), r0["cluster_avg"].copy())


# revision 19
# speedup vs baseline: 3.1966x; 1.0081x over previous
"""Trainium2 Bass kernel for nn_Cluster_assigner (vq_codebook).

Sharding: data-parallel over batch bs=64 -> 8 cores x 8 batches.
Per core:
  phase 1 (per batch b):
    x_emb[b] (n,d) = x[b].T @ W.T + bias   via PE (stationary = x tiles (s,v))
    scores.T/P_u.T (n,c) fused as extra moving operand [G.T|H.T] where
      G = cemb @ W, H = chat @ W  (chat = l2norm(cluster_emb))
    norm2[n] = sum_d x_emb^2 (DVE scalar_tensor_tensor accum_out)
    expT = exp(scores.T * 1/sqrt(d))  (softmax w/o max-sub: scores ~ N(0,1))
    P_sum (n,c) += P_u.T * inv_norm   (+ const2 * sum_b inv_norm at the end)
    x_emb spilled to DRAM scratch (read back in phase 2)
  AllReduce P_sum (128KB) -> sinkhorn -> prob_avg (output 1) -> mask
  phase 2 (per batch): A.T (n,c) = expT * invE * mask ; attn-out accumulated
    into one PSUM bank across all batches; AllReduce (64KB) -> /64 (output 2)
"""

import math
import sys

import numpy as np

for _p in ("/opt/trn_rl_repo",):
    if _p not in sys.path:
        sys.path.insert(0, _p)

import concourse.bass as bass  # noqa: E402
import concourse.tile as tile  # noqa: E402
from concourse import bacc, mybir  # noqa: E402
from concourse import bass_utils  # noqa: E402
from concourse.masks import make_identity  # noqa: E402

F32 = mybir.dt.float32
BF16 = mybir.dt.bfloat16
AF = mybir.ActivationFunctionType
ALU = mybir.AluOpType

N_CORES = 8
BS = 64
BS_L = BS // N_CORES          # 8 batches per core
S = 1024                      # seq_len (contraction for x_emb)
V = 1024                      # n_vars (= n in the notes)
D = 512                       # d_model
C = 32                        # n_cluster
ST = S // 128                 # 8 s-tiles
VT = V // 128                 # 8 v-tiles
DT = D // 128                 # 4 d-tiles
EPS = 0.05
TEMP = 0.07
ATT_SCALE = 1.0 / math.sqrt(float(D))
SINK_SCALE = 1.0 / (BS * EPS)

TRACE = False
LAST_RESULTS = None


def _newton_rsqrt(nc, pool, inv, n2, shape, tag):
    """One Newton step refining inv ~= 1/sqrt(n2): inv *= (1.5 - 0.5*n2*inv^2)."""
    t = pool.tile(shape, F32, tag=tag)
    nc.vector.tensor_mul(t, inv, inv)
    nc.vector.tensor_mul(t, t, n2)
    # t = -0.5*t + 1.5  (Copy: out = in*scale + bias)
    nc.scalar.activation(t, t, AF.Copy, bias=1.5, scale=-0.5)
    nc.vector.tensor_mul(inv, inv, t)


def _body(tc, x_in, w_in, wt_in, ce_in, b_in, noise_in, pa_out, ca_out,
          with_bias):
    nc = tc.nc

    from contextlib import ExitStack
    ctx = ExitStack()
    const = ctx.enter_context(tc.tile_pool(name="const", bufs=1))
    small = ctx.enter_context(tc.tile_pool(name="small", bufs=2))
    xpool = ctx.enter_context(tc.tile_pool(name="xp", bufs=2))
    xepool = ctx.enter_context(tc.tile_pool(name="xep", bufs=6))
    sppool = ctx.enter_context(tc.tile_pool(name="spp", bufs=2))
    persist = ctx.enter_context(tc.tile_pool(name="pers", bufs=1))
    dram = ctx.enter_context(tc.tile_pool(name="dram", bufs=1, space="DRAM"))

    # ---------------- first x tile load (ahead of setup DMAs) ----------
    x_tiles = {}

    def load_x(b):
        t = xpool.tile([128, ST, V], BF16, tag="x")
        nc.gpsimd.dma_start(out=t, in_=x_in[b].rearrange("(k p) v -> p k v", p=128))
        x_tiles[b] = t

    load_x(0)

    # ---------------- constants / setup ----------------
    ident = const.tile([128, 128], F32)
    make_identity(nc, ident)
    wt_sb = const.tile([128, ST, D], BF16)         # W.T tiles (s_p, s_t, d), bf16
    nc.gpsimd.dma_start(out=wt_sb, in_=wt_in.rearrange("(k p) d -> p k d", p=128))
    w_sb = const.tile([128, DT, S], F32)           # W tiles (d_p, d_t, s)
    nc.sync.dma_start(out=w_sb, in_=w_in.rearrange("(k p) s -> p k s", p=128))
    ce_sb = const.tile([C, D], F32)
    nc.sync.dma_start(out=ce_sb, in_=ce_in)
    if with_bias:
        b_row = const.tile([1, D], BF16)
        nc.gpsimd.dma_start(out=b_row, in_=b_in.rearrange("(a d) -> a d", a=1))
        b_col = const.tile([128, DT], F32)
        nc.sync.dma_start(out=b_col, in_=b_in.rearrange("(k p) -> p k", p=128))
    noise_sb = const.tile([128, VT, C], F32)
    nc.sync.dma_start(out=noise_sb, in_=noise_in.rearrange("(k p) c -> p k c", p=128))
    ones_row = const.tile([1, 128], BF16)
    nc.vector.memset(ones_row, 1.0)
    ones_col = const.tile([128, 1], BF16)
    nc.vector.memset(ones_col, 1.0)

    # chat = l2norm(cluster_emb)
    sq_c = small.tile([C, D], F32, tag="sqc")
    n2_c = small.tile([C, 1], F32, tag="n2c")
    nc.vector.scalar_tensor_tensor(
        out=sq_c, in0=ce_sb, scalar=1.0, in1=ce_sb,
        op0=ALU.mult, op1=ALU.mult, accum_out=n2_c)
    lg_c = small.tile([C, 1], F32, tag="nrmc")
    nc.scalar.activation(lg_c, n2_c, AF.Ln)
    inv_c = small.tile([C, 1], F32, tag="invc")
    nc.scalar.activation(inv_c, lg_c, AF.Exp, scale=-0.5)
    _newton_rsqrt(nc, small, inv_c, n2_c, [C, 1], "newc")
    chat_sb = const.tile([C, D], F32)
    nc.vector.tensor_scalar_mul(chat_sb, ce_sb, inv_c)

    # ce2 = [cemb.T | chat.T]  (d_p, d_t, 2C)
    ce2_sb = const.tile([128, DT, 2 * C], F32)
    with tc.tile_pool(name="ps_setup", bufs=2, space="PSUM") as ps_setup:
        for dk in range(DT):
            pst = ps_setup.tile([128, 2 * C], F32, tag="tr")
            nc.tensor.transpose(pst[:, 0:C], ce_sb[:, dk * 128:(dk + 1) * 128],
                                ident[:C, :C])
            nc.tensor.transpose(pst[:, C:2 * C], chat_sb[:, dk * 128:(dk + 1) * 128],
                                ident[:C, :C])
            nc.vector.tensor_copy(ce2_sb[:, dk, :], pst)

        # GH.T (s_p, s_t, 2C): GH.T[s, :] = [G.T | H.T], G = cemb@W, H = chat@W
        ght_sb = const.tile([128, ST, 2 * C], BF16)
        for st_i in range(ST):
            psg = ps_setup.tile([128, 2 * C], F32, tag="gh")
            for dk in range(DT):
                nc.tensor.matmul(
                    psg, w_sb[:, dk, st_i * 128:(st_i + 1) * 128], ce2_sb[:, dk, :],
                    start=(dk == 0), stop=(dk == DT - 1), skip_group_check=True)
            nc.vector.tensor_copy(ght_sb[:, st_i, :], psg)

        if with_bias:
            # const2[c] = chat @ b  (row layout (1, C))
            psc = ps_setup.tile([1, C], F32, tag="c2")
            for dk in range(DT):
                nc.tensor.matmul(psc, b_col[:, dk:dk + 1],
                                 ce2_sb[:, dk, C:2 * C],
                                 start=(dk == 0), stop=(dk == DT - 1),
                                 skip_group_check=True)
            const2_row = const.tile([1, C], F32)
            nc.vector.tensor_copy(const2_row, psc)
            # materialize across partitions (stride-0 only legal from DRAM)
            c2_dram = dram.tile([1, C], F32)
            nc.sync.dma_start(out=c2_dram, in_=const2_row)
            const2_bc = const.tile([128, C], F32)
            nc.gpsimd.dma_start(out=const2_bc, in_=c2_dram.to_broadcast([128, C]))

    # DRAM scratch for x_emb and collective bounce buffers
    xe_dram = dram.tile([BS_L, VT, 128, D], BF16)
    cc_p_in = dram.tile([128, VT, C], F32)
    cc_p_out = dram.tile([128, VT, C], F32)
    cc_a_in = dram.tile([C, D], F32)
    cc_a_out = dram.tile([C, D], F32)

    # persistent SBUF
    expT_sb = persist.tile([128, BS_L, VT, C], BF16)   # exp(scores.T*scale)
    PuT_all = persist.tile([128, BS_L, VT, C], F32)    # unnormalized prob.T
    n2_all = persist.tile([128, BS_L, VT], F32)        # row norms^2 of x_emb
    Psum_sb = persist.tile([128, VT, C], F32)          # sum_b P_u.T * inv_n
    Sinv_sb = persist.tile([128, VT], F32)             # sum_b inv_n (for const2)
    invE_sb = persist.tile([C, BS_L], F32)             # 1/softmax-denominator
    nc.vector.memset(Psum_sb, 0.0)
    nc.vector.memset(Sinv_sb, 0.0)

    ps_xe = ctx.enter_context(tc.tile_pool(name="ps_xe", bufs=2, space="PSUM"))
    ps_sp = ctx.enter_context(tc.tile_pool(name="ps_sp", bufs=2, space="PSUM"))
    ps_e = ctx.enter_context(tc.tile_pool(name="ps_e", bufs=2, space="PSUM"))

    # ---------------- phase 1 ----------------
    for b in range(BS_L):
        if b not in x_tiles:
            load_x(b)
        x_sb = x_tiles.pop(b)
        xe_sb = xepool.tile([128, VT, D], BF16, tag="xe")

        for vt in range(VT):
            pxe = ps_xe.tile([128, D], F32, tag="xe")
            psp = ps_sp.tile([128, 2 * C], F32, tag="sp")
            for st_i in range(ST):
                lhsT = x_sb[:, st_i, vt * 128:(vt + 1) * 128]
                nc.tensor.matmul(pxe, lhsT, wt_sb[:, st_i, :],
                                 start=(st_i == 0),
                                 stop=(not with_bias and st_i == ST - 1),
                                 skip_group_check=True)
                nc.tensor.matmul(psp, lhsT, ght_sb[:, st_i, :],
                                 start=(st_i == 0), stop=(st_i == ST - 1),
                                 skip_group_check=True)
            if with_bias:
                # bias add via K=1 matmul: xe += ones_col(v) * b_row(d)
                nc.tensor.matmul(pxe, ones_row, b_row, start=False, stop=True,
                                 skip_group_check=True)
            nc.vector.tensor_copy(xe_sb[:, vt, :], pxe)
            # norm2 = sum_d xe^2 straight from PSUM (fp32 accurate).
            # ACT Square: PSUM has a single DVE read port, so a 2-operand
            # DVE op reading pxe twice is illegal; ScalarE reads it once.
            sq_t = small.tile([128, D], F32, tag="sq")
            nc.scalar.activation(sq_t, pxe, AF.Square,
                                 accum_out=n2_all[:, b, vt:vt + 1])
            # keep unnormalized prob.T; exp(scores.T) straight from PSUM
            nc.vector.tensor_copy(PuT_all[:, b, vt, :], psp[:, C:2 * C])
            nc.scalar.activation(expT_sb[:, b, vt, :], psp[:, 0:C],
                                 AF.Exp, scale=ATT_SCALE)

        # spill x_emb to DRAM
        nc.sync.dma_start(out=xe_dram[b].rearrange("k p d -> p k d"), in_=xe_sb)

        # softmax denominators in column layout: E[c] = sum_n expT
        # (expT tile as stationary, ones as moving -> out (C, 1))
        pse = ps_e.tile([C, 1], F32, tag="E")
        for vt in range(VT):
            nc.tensor.matmul(pse, expT_sb[:, b, vt, :], ones_col,
                             start=(vt == 0), stop=(vt == VT - 1),
                             skip_group_check=True)
        esum = small.tile([C, 1], F32, tag="es")
        nc.vector.tensor_copy(esum, pse)
        nc.vector.reciprocal(invE_sb[:, b:b + 1], esum)

        # norms + P accumulation per batch (hidden under PE).
        # inv_n = exp(-0.5*ln(n2)) + Newton: Ln/Exp share one ACT table set,
        # so the per-batch chain never reloads tables (Sqrt would).
        lgn = small.tile([128, VT], F32, tag="nrm")
        nc.scalar.activation(lgn, n2_all[:, b, :], AF.Ln)
        invn = small.tile([128, VT], F32, tag="invn")
        nc.scalar.activation(invn, lgn, AF.Exp, scale=-0.5)
        _newton_rsqrt(nc, small, invn, n2_all[:, b, :], [128, VT], "newt")
        if with_bias:
            nc.vector.tensor_add(Sinv_sb, Sinv_sb, invn)
        for vt in range(VT):
            nc.vector.scalar_tensor_tensor(
                out=Psum_sb[:, vt, :], in0=PuT_all[:, b, vt, :],
                scalar=invn[:, vt:vt + 1], in1=Psum_sb[:, vt, :],
                op0=ALU.mult, op1=ALU.add)
    if with_bias:
        # finish P_sum: += const2 * Sinv
        for vt in range(VT):
            nc.vector.scalar_tensor_tensor(
                out=Psum_sb[:, vt, :], in0=const2_bc,
                scalar=Sinv_sb[:, vt:vt + 1], in1=Psum_sb[:, vt, :],
                op0=ALU.mult, op1=ALU.add)

    # ---------------- all-reduce P_sum ----------------
    nc.sync.dma_start(out=cc_p_in, in_=Psum_sb)
    nc.gpsimd.collective_compute(
        "AllReduce", ALU.add, replica_groups=[list(range(N_CORES))],
        ins=[cc_p_in.opt()], outs=[cc_p_out.opt()])
    Pg_sb = persist.tile([128, VT, C], F32)
    nc.sync.dma_start(out=Pg_sb, in_=cc_p_out)

    # ---------------- sinkhorn -> prob_avg (output 1) ----------------
    Q_sb = persist.tile([128, VT, C], F32)
    rs = small.tile([128, VT], F32, tag="rs")
    nc.scalar.activation(Q_sb, Pg_sb, AF.Exp, scale=SINK_SCALE)
    for vt in range(VT):
        nc.vector.reduce_sum(rs[:, vt:vt + 1], Q_sb[:, vt, :],
                             axis=mybir.AxisListType.X)
    irs = small.tile([128, VT], F32, tag="irs")
    nc.vector.reciprocal(irs, rs)
    probavg_sb = persist.tile([128, VT, C], F32)
    for vt in range(VT):
        nc.vector.tensor_scalar_mul(probavg_sb[:, vt, :], Q_sb[:, vt, :],
                                    irs[:, vt:vt + 1])
    nc.sync.dma_start(out=pa_out.rearrange("(k p) c -> p k c", p=128),
                      in_=probavg_sb)

    # ---------------- mask ----------------
    c_eps = small.tile([128, 1], F32, tag="ceps")
    nc.vector.memset(c_eps, 1e-10)
    c_1eps = small.tile([128, 1], F32, tag="c1eps")
    nc.vector.memset(c_1eps, 1.0 + 1e-10)
    c_one = small.tile([128, 1], F32, tag="cone")
    nc.vector.memset(c_one, 1.0)
    lp1 = small.tile([128, VT, C], F32, tag="lp1")
    nc.scalar.activation(lp1, probavg_sb, AF.Ln, bias=c_eps[:, 0:1])
    lp2 = small.tile([128, VT, C], F32, tag="lp2")
    nc.scalar.activation(lp2, probavg_sb, AF.Ln, bias=c_1eps[:, 0:1], scale=-1.0)
    nc.vector.tensor_sub(lp1, lp1, lp2)
    nc.vector.tensor_add(lp1, lp1, noise_sb)  # noise_sb = host logit_noise
    mask_sb = persist.tile([128, VT, C], BF16)
    nc.scalar.activation(mask_sb, lp1, AF.Sigmoid, scale=1.0 / TEMP)

    # ---------------- phase 2: attention ----------------
    osum = small.tile([C, D], F32, tag="os")
    nc.vector.memset(osum, 0.0)
    with tc.tile_pool(name="ps_out", bufs=2, space="PSUM") as ps_out:
        for b in range(BS_L):
            at_sb = small.tile([128, VT, C], BF16, tag="at")
            nc.vector.tensor_mul(at_sb, expT_sb[:, b, :, :], mask_sb)
            xeb = xepool.tile([128, VT, D], BF16, tag="xe")
            nc.sync.dma_start(out=xeb, in_=xe_dram[b].rearrange("k p d -> p k d"))
            pso = ps_out.tile([C, D], F32, tag="o")
            for vt in range(VT):
                nc.tensor.matmul(pso, at_sb[:, vt, :], xeb[:, vt, :],
                                 start=(vt == 0), stop=(vt == VT - 1),
                                 skip_group_check=True)
            # osum += pso * invE[b]  (per-partition scalar on cluster axis)
            nc.vector.scalar_tensor_tensor(
                out=osum, in0=pso, scalar=invE_sb[:, b:b + 1], in1=osum,
                op0=ALU.mult, op1=ALU.add)

    nc.sync.dma_start(out=cc_a_in, in_=osum)
    nc.gpsimd.collective_compute(
        "AllReduce", ALU.add, replica_groups=[list(range(N_CORES))],
        ins=[cc_a_in.opt()], outs=[cc_a_out.opt()])
    gsum = small.tile([C, D], F32, tag="gs")
    nc.sync.dma_start(out=gsum, in_=cc_a_out)
    final = small.tile([C, D], F32, tag="fin")
    nc.scalar.activation(final, gsum, AF.Copy, scale=1.0 / BS)
    nc.sync.dma_start(out=ca_out, in_=final)
    ctx.close()


_NC_CACHE = {}


def _get_nc(with_bias=False):
    if with_bias not in _NC_CACHE:
        nc = bacc.Bacc("TRN2", target_bir_lowering=False, debug=False,
                       enable_asserts=False, num_devices=N_CORES)
        x_in = nc.dram_tensor("x_l", [BS_L, S, V], F32, kind="ExternalInput").ap()
        w_in = nc.dram_tensor("w", [D, S], F32, kind="ExternalInput").ap()
        wt_in = nc.dram_tensor("wt", [S, D], F32, kind="ExternalInput").ap()
        ce_in = nc.dram_tensor("cemb", [C, D], F32, kind="ExternalInput").ap()
        b_in = nc.dram_tensor("bvec", [D], F32, kind="ExternalInput").ap()
        noise_in = nc.dram_tensor("noise", [V, C], F32, kind="ExternalInput").ap()
        pa_out = nc.dram_tensor("prob_avg", [V, C], F32, kind="ExternalOutput").ap()
        ca_out = nc.dram_tensor("cluster_avg", [C, D], F32,
                                kind="ExternalOutput").ap()
        with tile.TileContext(nc) as tc:
            _body(tc, x_in, w_in, wt_in, ce_in, b_in, noise_in, pa_out, ca_out,
                  with_bias)
        nc.compile()
        _NC_CACHE[with_bias] = nc
    return _NC_CACHE[with_bias]


def kernel(x, cluster_emb, W, b, noise):
    global LAST_RESULTS
    bv = np.ascontiguousarray(np.asarray(b, dtype=np.float32))
    nc = _get_nc(bool(np.any(bv)))
    x = np.ascontiguousarray(np.asarray(x, dtype=np.float32))
    W = np.ascontiguousarray(np.asarray(W, dtype=np.float32))
    wt = np.ascontiguousarray(W.T)
    ce = np.ascontiguousarray(np.asarray(cluster_emb, dtype=np.float32))
    nzf = np.asarray(noise, dtype=np.float32)
    nz = np.ascontiguousarray(np.log(nzf) - np.log1p(-nzf))
    in_maps = []
    for core in range(N_CORES):
        in_maps.append({
            "x_l": np.ascontiguousarray(x[core * BS_L:(core + 1) * BS_L]),
            "w": W, "wt": wt, "cemb": ce, "bvec": bv, "noise": nz,
        })
    res = bass_utils.run_bass_kernel_spmd(
        nc, in_maps, core_ids=list(range(N_CORES)), trace=TRACE)
    LAST_RESULTS = res
    r0 = res.results[0]
    return (r0["prob_avg"].copy(), r0["cluster_avg"].copy())


# revision 21
# speedup vs baseline: 3.2658x; 1.0216x over previous
"""Trainium2 Bass kernel for nn_Cluster_assigner (vq_codebook).

Sharding: data-parallel over batch bs=64 -> 8 cores x 8 batches.
Per core:
  phase 1 (per batch b):
    x_emb[b] (n,d) = x[b].T @ W.T + bias   via PE (stationary = x tiles (s,v))
    scores.T/P_u.T (n,c) fused as extra moving operand [G.T|H.T] where
      G = cemb @ W, H = chat @ W  (chat = l2norm(cluster_emb))
    norm2[n] = sum_d x_emb^2 (DVE scalar_tensor_tensor accum_out)
    expT = exp(scores.T * 1/sqrt(d))  (softmax w/o max-sub: scores ~ N(0,1))
    P_sum (n,c) += P_u.T * inv_norm   (+ const2 * sum_b inv_norm at the end)
    x_emb spilled to DRAM scratch (read back in phase 2)
  AllReduce P_sum (128KB) -> sinkhorn -> prob_avg (output 1) -> mask
  phase 2 (per batch): A.T (n,c) = expT * invE * mask ; attn-out accumulated
    into one PSUM bank across all batches; AllReduce (64KB) -> /64 (output 2)
"""

import math
import sys

import numpy as np

for _p in ("/opt/trn_rl_repo",):
    if _p not in sys.path:
        sys.path.insert(0, _p)

import concourse.bass as bass  # noqa: E402
import concourse.tile as tile  # noqa: E402
from concourse import bacc, mybir  # noqa: E402
from concourse import bass_utils  # noqa: E402
from concourse.masks import make_identity  # noqa: E402

F32 = mybir.dt.float32
BF16 = mybir.dt.bfloat16
AF = mybir.ActivationFunctionType
ALU = mybir.AluOpType

N_CORES = 8
BS = 64
BS_L = BS // N_CORES          # 8 batches per core
S = 1024                      # seq_len (contraction for x_emb)
V = 1024                      # n_vars (= n in the notes)
D = 512                       # d_model
C = 32                        # n_cluster
ST = S // 128                 # 8 s-tiles
VT = V // 128                 # 8 v-tiles
DT = D // 128                 # 4 d-tiles
EPS = 0.05
TEMP = 0.07
ATT_SCALE = 1.0 / math.sqrt(float(D))
SINK_SCALE = 1.0 / (BS * EPS)

TRACE = False
LAST_RESULTS = None


def _newton_rsqrt(nc, pool, inv, n2, shape, tag):
    """One Newton step refining inv ~= 1/sqrt(n2): inv *= (1.5 - 0.5*n2*inv^2)."""
    t = pool.tile(shape, F32, tag=tag)
    nc.vector.tensor_mul(t, inv, inv)
    nc.vector.tensor_mul(t, t, n2)
    # t = -0.5*t + 1.5  (Copy: out = in*scale + bias)
    nc.scalar.activation(t, t, AF.Copy, bias=1.5, scale=-0.5)
    nc.vector.tensor_mul(inv, inv, t)


def _body(tc, x_in, w_in, wt_in, ce_in, b_in, noise_in, pa_out, ca_out,
          with_bias):
    nc = tc.nc

    from contextlib import ExitStack
    ctx = ExitStack()
    const = ctx.enter_context(tc.tile_pool(name="const", bufs=1))
    small = ctx.enter_context(tc.tile_pool(name="small", bufs=2))
    xpool = ctx.enter_context(tc.tile_pool(name="xp", bufs=2))
    sppool = ctx.enter_context(tc.tile_pool(name="spp", bufs=2))
    persist = ctx.enter_context(tc.tile_pool(name="pers", bufs=1))
    dram = ctx.enter_context(tc.tile_pool(name="dram", bufs=1, space="DRAM"))

    # ---------------- first x tile load (ahead of setup DMAs) ----------
    x_tiles = {}

    def load_x(b):
        t = xpool.tile([128, ST, V], BF16, tag="x")
        nc.gpsimd.dma_start(out=t, in_=x_in[b].rearrange("(k p) v -> p k v", p=128))
        x_tiles[b] = t

    load_x(0)

    # ---------------- constants / setup ----------------
    ident = const.tile([128, 128], F32)
    make_identity(nc, ident)
    wt_sb = const.tile([128, ST, D], BF16)         # W.T tiles (s_p, s_t, d), bf16
    nc.gpsimd.dma_start(out=wt_sb, in_=wt_in.rearrange("(k p) d -> p k d", p=128))
    setup_cm = tc.tile_pool(name="setup", bufs=1)
    setup_pool = setup_cm.__enter__()
    w_sb = setup_pool.tile([128, DT, S], F32)      # W tiles (d_p, d_t, s)
    nc.sync.dma_start(out=w_sb, in_=w_in.rearrange("(k p) s -> p k s", p=128))
    ce_sb = const.tile([C, D], F32)
    nc.sync.dma_start(out=ce_sb, in_=ce_in)
    if with_bias:
        b_row = const.tile([1, D], BF16)
        nc.gpsimd.dma_start(out=b_row, in_=b_in.rearrange("(a d) -> a d", a=1))
        b_col = const.tile([128, DT], F32)
        nc.sync.dma_start(out=b_col, in_=b_in.rearrange("(k p) -> p k", p=128))
    noise_sb = const.tile([128, VT, C], F32)
    nc.sync.dma_start(out=noise_sb, in_=noise_in.rearrange("(k p) c -> p k c", p=128))
    ones_row = const.tile([1, 128], BF16)
    nc.vector.memset(ones_row, 1.0)
    ones_col = const.tile([128, 1], BF16)
    nc.vector.memset(ones_col, 1.0)

    # chat = l2norm(cluster_emb)
    sq_c = small.tile([C, D], F32, tag="sqc")
    n2_c = small.tile([C, 1], F32, tag="n2c")
    nc.vector.scalar_tensor_tensor(
        out=sq_c, in0=ce_sb, scalar=1.0, in1=ce_sb,
        op0=ALU.mult, op1=ALU.mult, accum_out=n2_c)
    lg_c = small.tile([C, 1], F32, tag="nrmc")
    nc.scalar.activation(lg_c, n2_c, AF.Ln)
    inv_c = small.tile([C, 1], F32, tag="invc")
    nc.scalar.activation(inv_c, lg_c, AF.Exp, scale=-0.5)
    _newton_rsqrt(nc, small, inv_c, n2_c, [C, 1], "newc")
    chat_sb = const.tile([C, D], F32)
    nc.vector.tensor_scalar_mul(chat_sb, ce_sb, inv_c)

    # ce2 = [cemb.T | chat.T]  (d_p, d_t, 2C)
    ce2_sb = const.tile([128, DT, 2 * C], F32)
    with tc.tile_pool(name="ps_setup", bufs=2, space="PSUM") as ps_setup:
        for dk in range(DT):
            pst = ps_setup.tile([128, 2 * C], F32, tag="tr")
            nc.tensor.transpose(pst[:, 0:C], ce_sb[:, dk * 128:(dk + 1) * 128],
                                ident[:C, :C])
            nc.tensor.transpose(pst[:, C:2 * C], chat_sb[:, dk * 128:(dk + 1) * 128],
                                ident[:C, :C])
            nc.vector.tensor_copy(ce2_sb[:, dk, :], pst)

        # GH.T (s_p, s_t, 2C): GH.T[s, :] = [G.T | H.T], G = cemb@W, H = chat@W
        ght_sb = const.tile([128, ST, 2 * C], BF16)
        for st_i in range(ST):
            psg = ps_setup.tile([128, 2 * C], F32, tag="gh")
            for dk in range(DT):
                nc.tensor.matmul(
                    psg, w_sb[:, dk, st_i * 128:(st_i + 1) * 128], ce2_sb[:, dk, :],
                    start=(dk == 0), stop=(dk == DT - 1), skip_group_check=True)
            nc.vector.tensor_copy(ght_sb[:, st_i, :], psg)

        if with_bias:
            # const2[c] = chat @ b  (row layout (1, C))
            psc = ps_setup.tile([1, C], F32, tag="c2")
            for dk in range(DT):
                nc.tensor.matmul(psc, b_col[:, dk:dk + 1],
                                 ce2_sb[:, dk, C:2 * C],
                                 start=(dk == 0), stop=(dk == DT - 1),
                                 skip_group_check=True)
            const2_row = const.tile([1, C], F32)
            nc.vector.tensor_copy(const2_row, psc)
            # materialize across partitions (stride-0 only legal from DRAM)
            c2_dram = dram.tile([1, C], F32)
            nc.sync.dma_start(out=c2_dram, in_=const2_row)
            const2_bc = const.tile([128, C], F32)
            nc.gpsimd.dma_start(out=const2_bc, in_=c2_dram.to_broadcast([128, C]))

    setup_cm.__exit__(None, None, None)

    # collective bounce buffers
    cc_p_in = dram.tile([128, VT, C], F32)
    cc_p_out = dram.tile([128, VT, C], F32)
    cc_a_in = dram.tile([C, D], F32)
    cc_a_out = dram.tile([C, D], F32)

    # persistent SBUF
    xe_all = persist.tile([128, BS_L, VT, D], BF16)    # resident x_emb (bf16)
    expT_sb = persist.tile([128, BS_L, VT, C], BF16)   # exp(scores.T*scale)
    PuT_all = persist.tile([128, BS_L, VT, C], F32)    # unnormalized prob.T
    n2_all = persist.tile([128, BS_L, VT], F32)        # row norms^2 of x_emb
    Psum_sb = persist.tile([128, VT, C], F32)          # sum_b P_u.T * inv_n
    Sinv_sb = persist.tile([128, VT], F32)             # sum_b inv_n (for const2)
    invE_sb = persist.tile([C, BS_L], F32)             # 1/softmax-denominator
    nc.vector.memset(Psum_sb, 0.0)
    nc.vector.memset(Sinv_sb, 0.0)

    ps_xe = ctx.enter_context(tc.tile_pool(name="ps_xe", bufs=2, space="PSUM"))
    ps_sp = ctx.enter_context(tc.tile_pool(name="ps_sp", bufs=2, space="PSUM"))
    ps_e = ctx.enter_context(tc.tile_pool(name="ps_e", bufs=2, space="PSUM"))

    # ---------------- phase 1 ----------------
    for b in range(BS_L):
        if b not in x_tiles:
            load_x(b)
        x_sb = x_tiles.pop(b)

        for vt in range(VT):
            pxe = ps_xe.tile([128, D], F32, tag="xe")
            psp = ps_sp.tile([128, 2 * C], F32, tag="sp")
            for st_i in range(ST):
                lhsT = x_sb[:, st_i, vt * 128:(vt + 1) * 128]
                nc.tensor.matmul(pxe, lhsT, wt_sb[:, st_i, :],
                                 start=(st_i == 0),
                                 stop=(not with_bias and st_i == ST - 1),
                                 skip_group_check=True)
                nc.tensor.matmul(psp, lhsT, ght_sb[:, st_i, :],
                                 start=(st_i == 0), stop=(st_i == ST - 1),
                                 skip_group_check=True)
            if with_bias:
                # bias add via K=1 matmul: xe += ones_col(v) * b_row(d)
                nc.tensor.matmul(pxe, ones_row, b_row, start=False, stop=True,
                                 skip_group_check=True)
            nc.vector.tensor_copy(xe_all[:, b, vt, :], pxe)
            # norm2 = sum_d xe^2 straight from PSUM (fp32 accurate).
            # ACT Square: PSUM has a single DVE read port, so a 2-operand
            # DVE op reading pxe twice is illegal; ScalarE reads it once.
            sq_t = small.tile([128, D], F32, tag="sq")
            nc.scalar.activation(sq_t, pxe, AF.Square,
                                 accum_out=n2_all[:, b, vt:vt + 1])
            # keep unnormalized prob.T; exp(scores.T) straight from PSUM
            nc.vector.tensor_copy(PuT_all[:, b, vt, :], psp[:, C:2 * C])
            nc.scalar.activation(expT_sb[:, b, vt, :], psp[:, 0:C],
                                 AF.Exp, scale=ATT_SCALE)

        # softmax denominators in column layout: E[c] = sum_n expT
        # (expT tile as stationary, ones as moving -> out (C, 1))
        pse = ps_e.tile([C, 1], F32, tag="E")
        for vt in range(VT):
            nc.tensor.matmul(pse, expT_sb[:, b, vt, :], ones_col,
                             start=(vt == 0), stop=(vt == VT - 1),
                             skip_group_check=True)
        esum = small.tile([C, 1], F32, tag="es")
        nc.vector.tensor_copy(esum, pse)
        nc.vector.reciprocal(invE_sb[:, b:b + 1], esum)

        # norms + P accumulation per batch (hidden under PE).
        # inv_n = exp(-0.5*ln(n2)) + Newton: Ln/Exp share one ACT table set,
        # so the per-batch chain never reloads tables (Sqrt would).
        lgn = small.tile([128, VT], F32, tag="nrm")
        nc.scalar.activation(lgn, n2_all[:, b, :], AF.Ln)
        invn = small.tile([128, VT], F32, tag="invn")
        nc.scalar.activation(invn, lgn, AF.Exp, scale=-0.5)
        _newton_rsqrt(nc, small, invn, n2_all[:, b, :], [128, VT], "newt")
        if with_bias:
            nc.vector.tensor_add(Sinv_sb, Sinv_sb, invn)
        for vt in range(VT):
            nc.vector.scalar_tensor_tensor(
                out=Psum_sb[:, vt, :], in0=PuT_all[:, b, vt, :],
                scalar=invn[:, vt:vt + 1], in1=Psum_sb[:, vt, :],
                op0=ALU.mult, op1=ALU.add)
    if with_bias:
        # finish P_sum: += const2 * Sinv
        for vt in range(VT):
            nc.vector.scalar_tensor_tensor(
                out=Psum_sb[:, vt, :], in0=const2_bc,
                scalar=Sinv_sb[:, vt:vt + 1], in1=Psum_sb[:, vt, :],
                op0=ALU.mult, op1=ALU.add)

    # ---------------- all-reduce P_sum ----------------
    nc.sync.dma_start(out=cc_p_in, in_=Psum_sb)
    nc.gpsimd.collective_compute(
        "AllReduce", ALU.add, replica_groups=[list(range(N_CORES))],
        ins=[cc_p_in.opt()], outs=[cc_p_out.opt()])
    Pg_sb = persist.tile([128, VT, C], F32)
    nc.sync.dma_start(out=Pg_sb, in_=cc_p_out)

    # ---------------- sinkhorn -> prob_avg (output 1) ----------------
    Q_sb = persist.tile([128, VT, C], F32)
    rs = small.tile([128, VT], F32, tag="rs")
    nc.scalar.activation(Q_sb, Pg_sb, AF.Exp, scale=SINK_SCALE)
    for vt in range(VT):
        nc.vector.reduce_sum(rs[:, vt:vt + 1], Q_sb[:, vt, :],
                             axis=mybir.AxisListType.X)
    irs = small.tile([128, VT], F32, tag="irs")
    nc.vector.reciprocal(irs, rs)
    probavg_sb = persist.tile([128, VT, C], F32)
    for vt in range(VT):
        nc.vector.tensor_scalar_mul(probavg_sb[:, vt, :], Q_sb[:, vt, :],
                                    irs[:, vt:vt + 1])
    nc.sync.dma_start(out=pa_out.rearrange("(k p) c -> p k c", p=128),
                      in_=probavg_sb)

    # ---------------- mask ----------------
    c_eps = small.tile([128, 1], F32, tag="ceps")
    nc.vector.memset(c_eps, 1e-10)
    c_1eps = small.tile([128, 1], F32, tag="c1eps")
    nc.vector.memset(c_1eps, 1.0 + 1e-10)
    c_one = small.tile([128, 1], F32, tag="cone")
    nc.vector.memset(c_one, 1.0)
    lp1 = small.tile([128, VT, C], F32, tag="lp1")
    nc.scalar.activation(lp1, probavg_sb, AF.Ln, bias=c_eps[:, 0:1])
    lp2 = small.tile([128, VT, C], F32, tag="lp2")
    nc.scalar.activation(lp2, probavg_sb, AF.Ln, bias=c_1eps[:, 0:1], scale=-1.0)
    nc.vector.tensor_sub(lp1, lp1, lp2)
    nc.vector.tensor_add(lp1, lp1, noise_sb)  # noise_sb = host logit_noise
    mask_sb = persist.tile([128, VT, C], BF16)
    nc.scalar.activation(mask_sb, lp1, AF.Sigmoid, scale=1.0 / TEMP)

    # ---------------- phase 2: attention ----------------
    osum = small.tile([C, D], F32, tag="os")
    nc.vector.memset(osum, 0.0)
    with tc.tile_pool(name="ps_out", bufs=2, space="PSUM") as ps_out:
        for b in range(BS_L):
            at_sb = small.tile([128, VT, C], BF16, tag="at")
            nc.vector.tensor_mul(at_sb, expT_sb[:, b, :, :], mask_sb)
            pso = ps_out.tile([C, D], F32, tag="o")
            for vt in range(VT):
                nc.tensor.matmul(pso, at_sb[:, vt, :], xe_all[:, b, vt, :],
                                 start=(vt == 0), stop=(vt == VT - 1),
                                 skip_group_check=True)
            # osum += pso * invE[b]  (per-partition scalar on cluster axis)
            nc.vector.scalar_tensor_tensor(
                out=osum, in0=pso, scalar=invE_sb[:, b:b + 1], in1=osum,
                op0=ALU.mult, op1=ALU.add)

    nc.sync.dma_start(out=cc_a_in, in_=osum)
    nc.gpsimd.collective_compute(
        "AllReduce", ALU.add, replica_groups=[list(range(N_CORES))],
        ins=[cc_a_in.opt()], outs=[cc_a_out.opt()])
    gsum = small.tile([C, D], F32, tag="gs")
    nc.sync.dma_start(out=gsum, in_=cc_a_out)
    final = small.tile([C, D], F32, tag="fin")
    nc.scalar.activation(final, gsum, AF.Copy, scale=1.0 / BS)
    nc.sync.dma_start(out=ca_out, in_=final)
    ctx.close()


_NC_CACHE = {}


def _get_nc(with_bias=False):
    if with_bias not in _NC_CACHE:
        nc = bacc.Bacc("TRN2", target_bir_lowering=False, debug=False,
                       enable_asserts=False, num_devices=N_CORES)
        x_in = nc.dram_tensor("x_l", [BS_L, S, V], F32, kind="ExternalInput").ap()
        w_in = nc.dram_tensor("w", [D, S], F32, kind="ExternalInput").ap()
        wt_in = nc.dram_tensor("wt", [S, D], F32, kind="ExternalInput").ap()
        ce_in = nc.dram_tensor("cemb", [C, D], F32, kind="ExternalInput").ap()
        b_in = nc.dram_tensor("bvec", [D], F32, kind="ExternalInput").ap()
        noise_in = nc.dram_tensor("noise", [V, C], F32, kind="ExternalInput").ap()
        pa_out = nc.dram_tensor("prob_avg", [V, C], F32, kind="ExternalOutput").ap()
        ca_out = nc.dram_tensor("cluster_avg", [C, D], F32,
                                kind="ExternalOutput").ap()
        with tile.TileContext(nc) as tc:
            _body(tc, x_in, w_in, wt_in, ce_in, b_in, noise_in, pa_out, ca_out,
                  with_bias)
        nc.compile()
        _NC_CACHE[with_bias] = nc
    return _NC_CACHE[with_bias]


def kernel(x, cluster_emb, W, b, noise):
    global LAST_RESULTS
    bv = np.ascontiguousarray(np.asarray(b, dtype=np.float32))
    nc = _get_nc(bool(np.any(bv)))
    x = np.ascontiguousarray(np.asarray(x, dtype=np.float32))
    W = np.ascontiguousarray(np.asarray(W, dtype=np.float32))
    wt = np.ascontiguousarray(W.T)
    ce = np.ascontiguousarray(np.asarray(cluster_emb, dtype=np.float32))
    nzf = np.asarray(noise, dtype=np.float32)
    nz = np.ascontiguousarray(np.log(nzf) - np.log1p(-nzf))
    in_maps = []
    for core in range(N_CORES):
        in_maps.append({
            "x_l": np.ascontiguousarray(x[core * BS_L:(core + 1) * BS_L]),
            "w": W, "wt": wt, "cemb": ce, "bvec": bv, "noise": nz,
        })
    res = bass_utils.run_bass_kernel_spmd(
        nc, in_maps, core_ids=list(range(N_CORES)), trace=TRACE)
    LAST_RESULTS = res
    r0 = res.results[0]
    return (r0["prob_avg"].copy(), r0["cluster_avg"].copy())


# revision 23
# speedup vs baseline: 3.5094x; 1.0746x over previous
"""Trainium2 Bass kernel for nn_Cluster_assigner (vq_codebook).

Sharding: data-parallel over batch bs=64 -> 8 cores x 8 batches.
Per core:
  phase 1 (per batch b):
    x_emb[b] (n,d) = x[b].T @ W.T + bias   via PE (stationary = x tiles (s,v))
    scores.T/P_u.T (n,c) fused as extra moving operand [G.T|H.T] where
      G = cemb @ W, H = chat @ W  (chat = l2norm(cluster_emb))
    norm2[n] = sum_d x_emb^2 (DVE scalar_tensor_tensor accum_out)
    expT = exp(scores.T * 1/sqrt(d))  (softmax w/o max-sub: scores ~ N(0,1))
    P_sum (n,c) += P_u.T * inv_norm   (+ const2 * sum_b inv_norm at the end)
    x_emb spilled to DRAM scratch (read back in phase 2)
  AllReduce P_sum (128KB) -> sinkhorn -> prob_avg (output 1) -> mask
  phase 2 (per batch): A.T (n,c) = expT * invE * mask ; attn-out accumulated
    into one PSUM bank across all batches; AllReduce (64KB) -> /64 (output 2)
"""

import math
import sys

import numpy as np

for _p in ("/opt/trn_rl_repo",):
    if _p not in sys.path:
        sys.path.insert(0, _p)

import concourse.bass as bass  # noqa: E402
import concourse.tile as tile  # noqa: E402
from concourse import bacc, mybir  # noqa: E402
from concourse import bass_utils  # noqa: E402
from concourse.masks import make_identity  # noqa: E402

F32 = mybir.dt.float32
BF16 = mybir.dt.bfloat16
AF = mybir.ActivationFunctionType
ALU = mybir.AluOpType

N_CORES = 8
BS = 64
BS_L = BS // N_CORES          # 8 batches per core
S = 1024                      # seq_len (contraction for x_emb)
V = 1024                      # n_vars (= n in the notes)
D = 512                       # d_model
C = 32                        # n_cluster
ST = S // 128                 # 8 s-tiles
VT = V // 128                 # 8 v-tiles
DT = D // 128                 # 4 d-tiles
EPS = 0.05
TEMP = 0.07
ATT_SCALE = 1.0 / math.sqrt(float(D))
SINK_SCALE = 1.0 / (BS * EPS)

TRACE = False
LAST_RESULTS = None


def _newton_rsqrt(nc, pool, inv, n2, shape, tag):
    """One Newton step refining inv ~= 1/sqrt(n2): inv *= (1.5 - 0.5*n2*inv^2)."""
    t = pool.tile(shape, F32, tag=tag)
    ts = t[:, :inv.shape[-1]] if list(t.shape) != list(inv.shape) else t
    nc.vector.tensor_mul(ts, inv, inv)
    nc.vector.tensor_mul(ts, ts, n2)
    # t = -0.5*t + 1.5  (Copy: out = in*scale + bias)
    nc.scalar.activation(ts, ts, AF.Copy, bias=1.5, scale=-0.5)
    nc.vector.tensor_mul(inv, inv, ts)


def _body(tc, x_in, w_in, wt_in, ce_in, b_in, noise_in, pa_out, ca_out,
          with_bias):
    nc = tc.nc

    from contextlib import ExitStack
    ctx = ExitStack()
    const = ctx.enter_context(tc.tile_pool(name="const", bufs=1))
    small = ctx.enter_context(tc.tile_pool(name="small", bufs=2))
    xpool = ctx.enter_context(tc.tile_pool(name="xp", bufs=2))
    sppool = ctx.enter_context(tc.tile_pool(name="spp", bufs=2))
    persist = ctx.enter_context(tc.tile_pool(name="pers", bufs=1))
    dram = ctx.enter_context(tc.tile_pool(name="dram", bufs=1, space="DRAM"))

    # ---------------- first x tile load (ahead of setup DMAs) ----------
    x_tiles = {}

    def load_x(b):
        t = xpool.tile([128, ST, V], BF16, tag="x")
        nc.gpsimd.dma_start(out=t, in_=x_in[b].rearrange("(k p) v -> p k v", p=128))
        x_tiles[b] = t

    load_x(0)

    # ---------------- constants / setup ----------------
    ident = const.tile([128, 128], F32)
    make_identity(nc, ident)
    wt_sb = const.tile([128, ST, D], BF16)         # W.T tiles (s_p, s_t, d), bf16
    nc.gpsimd.dma_start(out=wt_sb, in_=wt_in.rearrange("(k p) d -> p k d", p=128))
    setup_cm = tc.tile_pool(name="setup", bufs=1)
    setup_pool = setup_cm.__enter__()
    w_sb = setup_pool.tile([128, DT, S], F32)      # W tiles (d_p, d_t, s)
    nc.sync.dma_start(out=w_sb, in_=w_in.rearrange("(k p) s -> p k s", p=128))
    ce_sb = const.tile([C, D], F32)
    nc.sync.dma_start(out=ce_sb, in_=ce_in)
    if with_bias:
        b_row = const.tile([1, D], BF16)
        nc.gpsimd.dma_start(out=b_row, in_=b_in.rearrange("(a d) -> a d", a=1))
        b_col = const.tile([128, DT], F32)
        nc.sync.dma_start(out=b_col, in_=b_in.rearrange("(k p) -> p k", p=128))
    noise_sb = const.tile([128, VT, C], F32)
    nc.sync.dma_start(out=noise_sb, in_=noise_in.rearrange("(k p) c -> p k c", p=128))
    ones_row = const.tile([1, 128], BF16)
    nc.vector.memset(ones_row, 1.0)
    ones_col = const.tile([128, 1], BF16)
    nc.vector.memset(ones_col, 1.0)

    # chat = l2norm(cluster_emb)
    sq_c = small.tile([C, D], F32, tag="sqc")
    n2_c = small.tile([C, 1], F32, tag="n2c")
    nc.vector.scalar_tensor_tensor(
        out=sq_c, in0=ce_sb, scalar=1.0, in1=ce_sb,
        op0=ALU.mult, op1=ALU.mult, accum_out=n2_c)
    lg_c = small.tile([C, 1], F32, tag="nrmc")
    nc.scalar.activation(lg_c, n2_c, AF.Ln)
    inv_c = small.tile([C, 1], F32, tag="invc")
    nc.scalar.activation(inv_c, lg_c, AF.Exp, scale=-0.5)
    _newton_rsqrt(nc, small, inv_c, n2_c, [C, 1], "newc")
    chat_sb = const.tile([C, D], F32)
    nc.vector.tensor_scalar_mul(chat_sb, ce_sb, inv_c)

    # ce2 = [cemb.T | chat.T]  (d_p, d_t, 2C)
    ce2_sb = const.tile([128, DT, 2 * C], F32)
    with tc.tile_pool(name="ps_setup", bufs=2, space="PSUM") as ps_setup:
        for dk in range(DT):
            pst = ps_setup.tile([128, 2 * C], F32, tag="tr")
            nc.tensor.transpose(pst[:, 0:C], ce_sb[:, dk * 128:(dk + 1) * 128],
                                ident[:C, :C])
            nc.tensor.transpose(pst[:, C:2 * C], chat_sb[:, dk * 128:(dk + 1) * 128],
                                ident[:C, :C])
            nc.vector.tensor_copy(ce2_sb[:, dk, :], pst)

        # GH.T (s_p, s_t, 2C): GH.T[s, :] = [G.T | H.T], G = cemb@W, H = chat@W
        ght_sb = const.tile([128, ST, 2 * C], BF16)
        for st_i in range(ST):
            psg = ps_setup.tile([128, 2 * C], F32, tag="gh")
            for dk in range(DT):
                nc.tensor.matmul(
                    psg, w_sb[:, dk, st_i * 128:(st_i + 1) * 128], ce2_sb[:, dk, :],
                    start=(dk == 0), stop=(dk == DT - 1), skip_group_check=True)
            nc.vector.tensor_copy(ght_sb[:, st_i, :], psg)

        if with_bias:
            # const2[c] = chat @ b  (row layout (1, C))
            psc = ps_setup.tile([1, C], F32, tag="c2")
            for dk in range(DT):
                nc.tensor.matmul(psc, b_col[:, dk:dk + 1],
                                 ce2_sb[:, dk, C:2 * C],
                                 start=(dk == 0), stop=(dk == DT - 1),
                                 skip_group_check=True)
            const2_row = const.tile([1, C], F32)
            nc.vector.tensor_copy(const2_row, psc)
            # materialize across partitions (stride-0 only legal from DRAM)
            c2_dram = dram.tile([1, C], F32)
            nc.sync.dma_start(out=c2_dram, in_=const2_row)
            const2_bc = const.tile([128, C], F32)
            nc.gpsimd.dma_start(out=const2_bc, in_=c2_dram.to_broadcast([128, C]))

    setup_cm.__exit__(None, None, None)

    # collective bounce buffers
    cc_p_in = dram.tile([128, VT, C], F32)
    cc_p_out = dram.tile([128, VT, C], F32)

    # persistent SBUF
    xe_all = persist.tile([128, BS_L, VT, D], BF16)    # resident x_emb (bf16)
    expT_sb = persist.tile([128, BS_L, VT, C], BF16)   # exp(scores.T*scale)
    PuT_all = persist.tile([128, BS_L, VT, C], F32)    # unnormalized prob.T
    n2_all = persist.tile([128, BS_L, VT], F32)        # row norms^2 of x_emb
    Psum_sb = persist.tile([128, VT, C], F32)          # sum_b P_u.T * inv_n
    Sinv_sb = persist.tile([128, VT], F32)             # sum_b inv_n (for const2)
    invE_sb = persist.tile([C, BS_L], F32)             # 1/softmax-denominator
    nc.vector.memset(Psum_sb, 0.0)
    nc.vector.memset(Sinv_sb, 0.0)

    ps_xe = ctx.enter_context(tc.tile_pool(name="ps_xe", bufs=2, space="PSUM"))
    ps_sp = ctx.enter_context(tc.tile_pool(name="ps_sp", bufs=2, space="PSUM"))
    ps_e = ctx.enter_context(tc.tile_pool(name="ps_e", bufs=2, space="PSUM"))

    # ---------------- phase 1 ----------------
    for b in range(BS_L):
        if b not in x_tiles:
            load_x(b)
        x_sb = x_tiles.pop(b)

        for vt in range(VT):
            pxe = ps_xe.tile([128, D], F32, tag="xe")
            psp = ps_sp.tile([128, 2 * C], F32, tag="sp")
            for st_i in range(ST):
                lhsT = x_sb[:, st_i, vt * 128:(vt + 1) * 128]
                nc.tensor.matmul(pxe, lhsT, wt_sb[:, st_i, :],
                                 start=(st_i == 0),
                                 stop=(not with_bias and st_i == ST - 1),
                                 skip_group_check=True)
                nc.tensor.matmul(psp, lhsT, ght_sb[:, st_i, :],
                                 start=(st_i == 0), stop=(st_i == ST - 1),
                                 skip_group_check=True)
            if with_bias:
                # bias add via K=1 matmul: xe += ones_col(v) * b_row(d)
                nc.tensor.matmul(pxe, ones_row, b_row, start=False, stop=True,
                                 skip_group_check=True)
            nc.vector.tensor_copy(xe_all[:, b, vt, :], pxe)
            # norm2 = sum_d xe^2 straight from PSUM (fp32 accurate).
            # ACT Square: PSUM has a single DVE read port, so a 2-operand
            # DVE op reading pxe twice is illegal; ScalarE reads it once.
            sq_t = small.tile([128, D], F32, tag="sq")
            nc.scalar.activation(sq_t, pxe, AF.Square,
                                 accum_out=n2_all[:, b, vt:vt + 1])
            # keep unnormalized prob.T; exp(scores.T) straight from PSUM
            nc.vector.tensor_copy(PuT_all[:, b, vt, :], psp[:, C:2 * C])
            nc.scalar.activation(expT_sb[:, b, vt, :], psp[:, 0:C],
                                 AF.Exp, scale=ATT_SCALE)

        # softmax denominators in column layout: E[c] = sum_n expT
        # (expT tile as stationary, ones as moving -> out (C, 1))
        pse = ps_e.tile([C, 1], F32, tag="E")
        for vt in range(VT):
            nc.tensor.matmul(pse, expT_sb[:, b, vt, :], ones_col,
                             start=(vt == 0), stop=(vt == VT - 1),
                             skip_group_check=True)
        esum = small.tile([C, 1], F32, tag="es")
        nc.vector.tensor_copy(esum, pse)
        nc.vector.reciprocal(invE_sb[:, b:b + 1], esum)

        # norms + P accumulation (hidden under PE). For the last batch run
        # the chain per-vt so it overlaps the remaining matmuls instead of
        # serializing ahead of the P all-reduce.
        vt_groups = ([list(range(VT))] if b < BS_L - 1
                     else [[vt] for vt in range(VT)])
        for grp in vt_groups:
            v0, v1 = grp[0], grp[-1] + 1
            lgn = small.tile([128, VT], F32, tag="nrm")
            nc.scalar.activation(lgn[:, v0:v1], n2_all[:, b, v0:v1], AF.Ln)
            invn = small.tile([128, VT], F32, tag="invn")
            nc.scalar.activation(invn[:, v0:v1], lgn[:, v0:v1], AF.Exp,
                                 scale=-0.5)
            _newton_rsqrt(nc, small, invn[:, v0:v1], n2_all[:, b, v0:v1],
                          [128, VT], "newt")
            if with_bias:
                nc.vector.tensor_add(Sinv_sb[:, v0:v1], Sinv_sb[:, v0:v1],
                                     invn[:, v0:v1])
            for vt in grp:
                nc.vector.scalar_tensor_tensor(
                    out=Psum_sb[:, vt, :], in0=PuT_all[:, b, vt, :],
                    scalar=invn[:, vt:vt + 1], in1=Psum_sb[:, vt, :],
                    op0=ALU.mult, op1=ALU.add)
    if with_bias:
        # finish P_sum: += const2 * Sinv
        for vt in range(VT):
            nc.vector.scalar_tensor_tensor(
                out=Psum_sb[:, vt, :], in0=const2_bc,
                scalar=Sinv_sb[:, vt:vt + 1], in1=Psum_sb[:, vt, :],
                op0=ALU.mult, op1=ALU.add)

    # ---------------- all-reduce P_sum ----------------
    nc.sync.dma_start(out=cc_p_in, in_=Psum_sb)
    nc.gpsimd.collective_compute(
        "AllReduce", ALU.add, replica_groups=[list(range(N_CORES))],
        ins=[cc_p_in.opt()], outs=[cc_p_out.opt()])
    Pg_sb = persist.tile([128, VT, C], F32)
    nc.sync.dma_start(out=Pg_sb, in_=cc_p_out)

    # ---------------- sinkhorn -> prob_avg (output 1) ----------------
    Q_sb = persist.tile([128, VT, C], F32)
    rs = small.tile([128, VT], F32, tag="rs")
    nc.scalar.activation(Q_sb, Pg_sb, AF.Exp, scale=SINK_SCALE)
    for vt in range(VT):
        nc.vector.reduce_sum(rs[:, vt:vt + 1], Q_sb[:, vt, :],
                             axis=mybir.AxisListType.X)
    irs = small.tile([128, VT], F32, tag="irs")
    nc.vector.reciprocal(irs, rs)
    probavg_sb = persist.tile([128, VT, C], F32)
    for vt in range(VT):
        nc.vector.tensor_scalar_mul(probavg_sb[:, vt, :], Q_sb[:, vt, :],
                                    irs[:, vt:vt + 1])
    nc.sync.dma_start(out=pa_out.rearrange("(k p) c -> p k c", p=128),
                      in_=probavg_sb)

    # ---------------- mask ----------------
    c_eps = small.tile([128, 1], F32, tag="ceps")
    nc.vector.memset(c_eps, 1e-10)
    c_1eps = small.tile([128, 1], F32, tag="c1eps")
    nc.vector.memset(c_1eps, 1.0 + 1e-10)
    c_one = small.tile([128, 1], F32, tag="cone")
    nc.vector.memset(c_one, 1.0)
    lp1 = small.tile([128, VT, C], F32, tag="lp1")
    nc.scalar.activation(lp1, probavg_sb, AF.Ln, bias=c_eps[:, 0:1])
    lp2 = small.tile([128, VT, C], F32, tag="lp2")
    nc.scalar.activation(lp2, probavg_sb, AF.Ln, bias=c_1eps[:, 0:1], scale=-1.0)
    nc.vector.tensor_sub(lp1, lp1, lp2)
    nc.vector.tensor_add(lp1, lp1, noise_sb)  # noise_sb = host logit_noise
    mask_sb = persist.tile([128, VT, C], BF16)
    nc.scalar.activation(mask_sb, lp1, AF.Sigmoid, scale=1.0 / TEMP)

    # ---------------- phase 2: attention ----------------
    osum = small.tile([C, D], F32, tag="os")
    nc.vector.memset(osum, 0.0)
    with tc.tile_pool(name="ps_out", bufs=2, space="PSUM") as ps_out:
        for b in range(BS_L):
            at_sb = small.tile([128, VT, C], BF16, tag="at")
            nc.vector.tensor_mul(at_sb, expT_sb[:, b, :, :], mask_sb)
            pso = ps_out.tile([C, D], F32, tag="o")
            for vt in range(VT):
                nc.tensor.matmul(pso, at_sb[:, vt, :], xe_all[:, b, vt, :],
                                 start=(vt == 0), stop=(vt == VT - 1),
                                 skip_group_check=True)
            # osum += pso * invE[b]  (per-partition scalar on cluster axis)
            nc.vector.scalar_tensor_tensor(
                out=osum, in0=pso, scalar=invE_sb[:, b:b + 1], in1=osum,
                op0=ALU.mult, op1=ALU.add)

    # local partial only -- host sums the 8 cores' outputs and divides by BS
    nc.sync.dma_start(out=ca_out, in_=osum)
    ctx.close()


_NC_CACHE = {}


def _get_nc(with_bias=False):
    if with_bias not in _NC_CACHE:
        nc = bacc.Bacc("TRN2", target_bir_lowering=False, debug=False,
                       enable_asserts=False, num_devices=N_CORES)
        x_in = nc.dram_tensor("x_l", [BS_L, S, V], F32, kind="ExternalInput").ap()
        w_in = nc.dram_tensor("w", [D, S], F32, kind="ExternalInput").ap()
        wt_in = nc.dram_tensor("wt", [S, D], F32, kind="ExternalInput").ap()
        ce_in = nc.dram_tensor("cemb", [C, D], F32, kind="ExternalInput").ap()
        b_in = nc.dram_tensor("bvec", [D], F32, kind="ExternalInput").ap()
        noise_in = nc.dram_tensor("noise", [V, C], F32, kind="ExternalInput").ap()
        pa_out = nc.dram_tensor("prob_avg", [V, C], F32, kind="ExternalOutput").ap()
        ca_out = nc.dram_tensor("cluster_avg", [C, D], F32,
                                kind="ExternalOutput").ap()
        with tile.TileContext(nc) as tc:
            _body(tc, x_in, w_in, wt_in, ce_in, b_in, noise_in, pa_out, ca_out,
                  with_bias)
        nc.compile()
        _NC_CACHE[with_bias] = nc
    return _NC_CACHE[with_bias]


def kernel(x, cluster_emb, W, b, noise):
    global LAST_RESULTS
    bv = np.ascontiguousarray(np.asarray(b, dtype=np.float32))
    nc = _get_nc(bool(np.any(bv)))
    x = np.ascontiguousarray(np.asarray(x, dtype=np.float32))
    W = np.ascontiguousarray(np.asarray(W, dtype=np.float32))
    wt = np.ascontiguousarray(W.T)
    ce = np.ascontiguousarray(np.asarray(cluster_emb, dtype=np.float32))
    nzf = np.asarray(noise, dtype=np.float32)
    nz = np.ascontiguousarray(np.log(nzf) - np.log1p(-nzf))
    in_maps = []
    for core in range(N_CORES):
        in_maps.append({
            "x_l": np.ascontiguousarray(x[core * BS_L:(core + 1) * BS_L]),
            "w": W, "wt": wt, "cemb": ce, "bvec": bv, "noise": nz,
        })
    res = bass_utils.run_bass_kernel_spmd(
        nc, in_maps, core_ids=list(range(N_CORES)), trace=TRACE)
    LAST_RESULTS = res
    prob_avg = res.results[0]["prob_avg"].copy()
    cluster = np.zeros((C, D), np.float64)
    for r in res.results:
        cluster += r["cluster_avg"].astype(np.float64)
    cluster_avg = (cluster / BS).astype(np.float32)
    return (prob_avg, cluster_avg)
